# revision 15
# baseline (speedup 1.0000x reference)
"""Trainium2 Bass kernel for a dense decoder layer (RMSNorm -> GQA attn -> RMSNorm -> SwiGLU MLP).

Sharding: token-parallel across 8 cores (no collectives). Each core owns 512
query tokens of one batch (two causally balanced 256-token blocks) and computes
K/V for its batch's full 1024 tokens. Host permutes the batch's token columns
per core so q tokens always sit at columns [256:768) and block A's causal
prefix fits in kv columns [0:512) -- the compiled program is identical on all
cores. Activations are feature-major [feature_part, token_free] so matmuls
chain with no transposes; V is produced token-major by swapping matmul operand
roles.

Precision: Q/K/V/O projections run in fp8e4 (DoubleRow perf mode, 2 k-tiles
per matmul) with fp32 PSUM accumulation; x and those four weights are
quantized host-side with per-tensor absmax scales. The dequant constant
1/(s_x*s_w) is folded into the rmsnorm-1 rsqrt (imm/eps divided by c^2), so
d1b carries it into the rope tables and V's per-token scale for free. The
attention output is written as fp8 scaled by s_a=32 (exact in bf16), with
1/s_a folded into the all-ones stats/denominator matmul weight and the
residual add carrying 1/(s_a*s_wo). Scores/PV and the whole MLP stay bf16
(fp8 there busts the 2e-2 error budget). Runtime-dependent dequant scalars
ride in a tiny "consts" input tensor since immediates are compile-time.

Softmax skips max-subtraction (|scores| < ~6) and causality is a host-built
0/1 mask multiplied after exp. Partition-axis reductions use an all-[1/32]
[128,128] stationary matmul, which also broadcasts results across partitions.
"""

import os

import numpy as np
import ml_dtypes

import concourse.bass as bass  # noqa: F401
import concourse.mybir as mybir
import concourse.tile as tile
from concourse import bacc
from concourse.bass_utils import run_bass_kernel_spmd

# ---- problem shapes (hardcoded) ----
B, S, H = 4, 1024, 2048
NH, KVH, HD = 16, 4, 128
I = 8192
EPS = 1e-6

P = 128
KT = H // P            # 16 k-tiles over H
T = 512                # q tokens per core
SKV = 1024             # kv tokens per core (its batch's full sequence)
DV = KVH * HD          # 512
REP = NH // KVH
N_CORES = 8
BLK = 256              # q block size
KVT_A, KVT_B = 4, 8    # kv tiles processed for block A / block B
NMSK = KVT_A + KVT_B   # 12
IT = I // P            # 64 k-tiles over I
Q0 = 256               # q tokens live at columns [Q0 : Q0+T)
SCALE = 1.0 / float(np.sqrt(HD))
SA = 32.0              # attn fp8 scale; 1/32 is exact in bf16

F32 = mybir.dt.float32
BF16 = mybir.dt.bfloat16
FP8 = mybir.dt.float8e4
MUL = mybir.AluOpType.mult
ADD = mybir.AluOpType.add
AFT = mybir.ActivationFunctionType
DR = mybir.MatmulPerfMode.DoubleRow

BF = ml_dtypes.bfloat16
F8 = ml_dtypes.float8_e4m3fn

LAST_RESULT = None  # BassKernelResults of the most recent run (for test harness)


def _install_ntff_hook():
    """The agent image's `antenv` lacks `axon_hooks`, so the boot shim's NTFF
    hook registration degrades silently and bass_utils crashes on import of
    antenv.axon_hooks when trace=True. Recreate the module and register the
    ctypes-based hook from trn_agent_boot."""
    import sys
    import types
    try:
        import antenv.axon_hooks  # noqa: F401
        return
    except ImportError:
        pass
    try:
        import antenv
    except ImportError:
        return
    mod = types.ModuleType("antenv.axon_hooks")
    _hook = [None]
    mod.set_axon_ntff_profile_hook = lambda h: _hook.__setitem__(0, h)
    mod.get_axon_ntff_profile_hook = lambda: _hook[0]
    sys.modules["antenv.axon_hooks"] = mod
    antenv.axon_hooks = mod
    try:
        from trn_agent_boot import trn_boot
        import ctypes
        so_path = "/opt/axon/libaxon_pjrt.so"
        lib = ctypes.CDLL(so_path)
        if hasattr(lib, "axon_start_nrt_profile"):
            mod.set_axon_ntff_profile_hook(
                trn_boot._ntff_profile_via_ctypes(so_path))
    except Exception:
        pass


_install_ntff_hook()


def build_nc():
    nc = bacc.Bacc(
        "TRN2",
        target_bir_lowering=False,
        debug=False,
        enable_asserts=False,
        num_devices=N_CORES,
    )

    # ---- DRAM I/O ----
    d_xkv = nc.dram_tensor("x_kv", [P, KT, SKV], FP8, kind="ExternalInput")
    d_xqres = nc.dram_tensor("x_qres", [P, KT, T], F32, kind="ExternalInput")
    d_ckv = nc.dram_tensor("ckv", [P, SKV], F32, kind="ExternalInput")
    d_skv = nc.dram_tensor("skv", [P, SKV], F32, kind="ExternalInput")
    d_consts = nc.dram_tensor("consts", [P, 4], F32, kind="ExternalInput")
    d_mask = nc.dram_tensor("mask", [P, NMSK, BLK], BF16, kind="ExternalInput")
    d_ones = nc.dram_tensor("ones_pp", [P, P], BF16, kind="ExternalInput")
    d_eye = nc.dram_tensor("eye_pp", [P, P], F32, kind="ExternalInput")
    d_wq = nc.dram_tensor("wq", [4, P, KT, 512], FP8, kind="ExternalInput")
    d_wk = nc.dram_tensor("wk", [P, KT, 512], FP8, kind="ExternalInput")
    d_wv = nc.dram_tensor("wv", [P, KT, 512], FP8, kind="ExternalInput")
    d_wo = nc.dram_tensor("wo", [4, P, KT, 512], FP8, kind="ExternalInput")
    d_wgu = nc.dram_tensor("wgu", [32, P, KT, 512], BF16, kind="ExternalInput")
    d_wd = nc.dram_tensor("wd", [16, P, IT, P], BF16, kind="ExternalInput")
    d_y = nc.dram_tensor("y", [P, KT, T], F32, kind="ExternalOutput")

    with tile.TileContext(nc) as tc:
        # long-lived pools
        glob_cm = tc.tile_pool(name="glob", bufs=1)
        gp = glob_cm.__enter__()
        wp_cm = tc.tile_pool(name="wstream", bufs=2)
        wp = wp_cm.__enter__()
        tp_cm = tc.tile_pool(name="tmp", bufs=2)
        tp = tp_cm.__enter__()
        wp8_cm = tc.tile_pool(name="wstream8", bufs=2)
        wp8 = wp8_cm.__enter__()

        ones_pp = gp.tile([P, P], BF16, tag="ones")
        consts = gp.tile([P, 4], F32, tag="consts")
        d2b = gp.tile([P, T], F32, tag="d2b")

        # ---- Phase 0: RMSNorm1 stats (projections run on raw fp8 x; the
        # per-token scale d1 is applied post-matmul, so the PE never waits
        # for the stats chain) ----
        pA_cm = tc.tile_pool(name="ph01", bufs=1)
        pA = pA_cm.__enter__()
        ps01_cm = tc.tile_pool(name="ps01", bufs=1, space="PSUM")
        ps01 = ps01_cm.__enter__()

        ckv_sb = pA.tile([P, SKV], F32, tag="ckv")
        skv_sb = pA.tile([P, SKV], F32, tag="skv")
        x_bf = pA.tile([P, KT, SKV], FP8, tag="xbf")
        d1b = pA.tile([P, SKV], F32, tag="d1b")
        d1s = pA.tile([P, SKV], F32, tag="d1s")
        d1t = pA.tile([P, SKV // P], F32, tag="d1t")   # token-major d1

        s1a = ps01.tile([P, 512], F32, tag="s1a")
        s1b = ps01.tile([P, 512], F32, tag="s1b")
        # x chunk 0 gates the whole stats chain -- DMA it before everything else
        nc.sync.dma_start(x_bf[:, 0, :], d_xkv[:, 0, :])
        nc.sync.dma_start(ones_pp[:], d_ones[:])
        nc.sync.dma_start(consts[:], d_consts[:])
        for k in range(KT):
            if k > 0:
                nc.sync.dma_start(x_bf[:, k, :], d_xkv[:, k, :])
            sqv = tp.tile([P, SKV], BF16, tag="sqkv")
            nc.vector.tensor_mul(out=sqv[:], in0=x_bf[:, k, :], in1=x_bf[:, k, :])
            nc.tensor.matmul(s1a[:], ones_pp[:], sqv[:, 0:512],
                             start=(k == 0), stop=(k == KT - 1))
            nc.tensor.matmul(s1b[:], ones_pp[:], sqv[:, 512:1024],
                             start=(k == 0), stop=(k == KT - 1))
        # tables DMA'd after the x stream so they don't delay the first matmuls
        nc.sync.dma_start(ckv_sb[:], d_ckv[:])
        nc.sync.dma_start(skv_sb[:], d_skv[:])
        # d1b = c_qk/sqrt(s/H + eps): the fp8 dequant constant is folded into
        # the sqrt scale/bias (divided by c^2), so no extra op is needed.
        # consts[:,0] = SA/(H*s_x^2*c^2), consts[:,1] = EPS/c^2.
        for half, ps in ((0, s1a), (1, s1b)):
            sl = slice(half * 512, (half + 1) * 512)
            nc.scalar.activation(d1s[:, sl], ps[:], AFT.Sqrt,
                                 bias=consts[:, 1:2], scale=consts[:, 0:1])
            scr = tp.tile([P, 512], F32, tag="d1scr")
            nc.vector.reciprocal_approx_accurate(out=d1b[:, sl], in_=d1s[:, sl],
                                                 scratch=scr[:])
        ps01_cm.__exit__(None, None, None)
        # fold the per-token d1*c into the rope tables (saves one DVE mul per tile)
        nc.vector.tensor_mul(out=ckv_sb[:], in0=ckv_sb[:], in1=d1b[:])
        nc.vector.tensor_mul(out=skv_sb[:], in0=skv_sb[:], in1=d1b[:])
        eye_sb = pA.tile([P, P], F32, tag="eye")
        nc.sync.dma_start(eye_sb[:], d_eye[:])

        psmm1_cm = tc.tile_pool(name="psmm1", bufs=6, space="PSUM")
        psmm1 = psmm1_cm.__enter__()

        # ---- Phase 1: Q/K/V projections (+rope), fp8 DoubleRow ----
        qkv_cm = tc.tile_pool(name="qkv", bufs=1, side="right")
        qp_ = qkv_cm.__enter__()
        q_fm = qp_.tile([P, NH, T], BF16, tag="qfm")
        k_fm = qp_.tile([P, KVH, SKV], BF16, tag="kfm")
        v_tm = qp_.tile([P, SKV // P, DV], BF16, tag="vtm")

        def rope_out(ps, cos_t, sin_t, out_ap, n):
            # out = raw*cos' + swap_halves(raw)*sin'  (d1, dequant + sin sign
            # pre-folded into the tables). The cos term reads PSUM directly so
            # the psum tile is released after [copy, mul] and never waits for
            # the swap DMA round-trip.
            raw = tp.tile([P, n], BF16, tag="rope_raw")
            nc.scalar.copy(raw[:], ps[:])
            rawc = tp.tile([P, n], F32, tag="rope_rc")
            nc.vector.tensor_mul(out=rawc[:], in0=raw[:], in1=cos_t)
            sw = tp.tile([P, n], BF16, tag="rope_sw")
            nc.sync.dma_start(sw[0:64, :], raw[64:128, :])
            nc.sync.dma_start(sw[64:128, :], raw[0:64, :])
            nc.vector.tensor_mul(out=sw[:], in0=sw[:], in1=sin_t)
            nc.vector.tensor_add(out=out_ap, in0=rawc[:], in1=sw[:])

        # Q: 16 heads; q tokens are x_bf columns [Q0 : Q0+T)
        for mb in range(4):
            wt = wp8.tile([P, KT, 512], FP8, tag="wchunk")
            nc.sync.dma_start(wt[:], d_wq[mb])
            for ms in range(4):
                h = mb * 4 + ms
                ps = psmm1.tile([P, T], F32, tag="mm")
                for k in range(0, KT, 2):
                    nc.tensor.matmul(ps[:], wt[:, k:k + 2, ms * P:(ms + 1) * P],
                                     x_bf[:, k:k + 2, Q0:Q0 + T],
                                     start=(k == 0), stop=(k == KT - 2),
                                     perf_mode=DR)
                rope_out(ps, ckv_sb[:, Q0:Q0 + T], skv_sb[:, Q0:Q0 + T],
                         q_fm[:, h, :], T)

        # K: 4 kv heads x 2 halves of the kv sequence
        wtk = wp8.tile([P, KT, 512], FP8, tag="wchunk")
        nc.sync.dma_start(wtk[:], d_wk[:])
        for kvh in range(KVH):
            for half in range(2):
                ps = psmm1.tile([P, T], F32, tag="mm")
                for k in range(0, KT, 2):
                    nc.tensor.matmul(ps[:], wtk[:, k:k + 2, kvh * P:(kvh + 1) * P],
                                     x_bf[:, k:k + 2, half * 512:(half + 1) * 512],
                                     start=(k == 0), stop=(k == KT - 2),
                                     perf_mode=DR)
                rope_out(ps, ckv_sb[:, half * 512:(half + 1) * 512],
                         skv_sb[:, half * 512:(half + 1) * 512],
                         k_fm[:, kvh, half * 512:(half + 1) * 512], 512)

        # token-major d1 (for scaling V rows): PE-transpose each 128-col block
        # of the (row-broadcast) d1b and keep one column. Runs here (not in
        # phase 0) so the Q/K matmuls above aren't head-of-line blocked on the
        # d1 chain.
        for kvt in range(SKV // P):
            tps = psmm1.tile([P, P], F32, tag="tr", bufs=2)
            nc.tensor.transpose(tps[:], d1b[:, kvt * P:(kvt + 1) * P], eye_sb[:])
            nc.scalar.copy(d1t[:, kvt:kvt + 1], tps[:, 0:1])

        # V: token-major directly (lhsT = activations, rhs = weights)
        wtv = wp8.tile([P, KT, 512], FP8, tag="wchunk")
        nc.sync.dma_start(wtv[:], d_wv[:])
        for kvt in range(SKV // P):
            ps = psmm1.tile([P, DV], F32, tag="mm")
            for k in range(0, KT, 2):
                nc.tensor.matmul(ps[:], x_bf[:, k:k + 2, kvt * P:(kvt + 1) * P],
                                 wtv[:, k:k + 2, :],
                                 start=(k == 0), stop=(k == KT - 2),
                                 perf_mode=DR)
            # rows are tokens: apply per-token d1*c as a per-partition ACT scale
            nc.scalar.mul(v_tm[:, kvt, :], ps[:], d1t[:, kvt:kvt + 1])

        pA_cm.__exit__(None, None, None)
        psmm1_cm.__exit__(None, None, None)

        # ---- Phase 2: attention ----
        attn_cm = tc.tile_pool(name="attn", bufs=1)
        ap_ = attn_cm.__enter__()
        exp_cm = tc.tile_pool(name="exp", bufs=2)
        ep = exp_cm.__enter__()
        ps2_cm = tc.tile_pool(name="ps2", bufs=1, space="PSUM")
        ps2 = ps2_cm.__enter__()

        mask_sb = ap_.tile([P, NMSK, BLK], BF16, tag="mask")
        nc.sync.dma_start(mask_sb[:], d_mask[:])
        attn_cat = ap_.tile([P, NH, T], FP8, tag="attncat")

        # software pipeline: scores/exp of chain i+1 are emitted before the
        # PV/denominator matmuls of chain i, so the PE never head-of-line
        # blocks on the ACT exp latency
        def emit_sc(h, b):
            kvh = h // REP
            nkv = KVT_A if b == 0 else KVT_B
            moff = 0 if b == 0 else KVT_A
            qs = q_fm[:, h, b * BLK:(b + 1) * BLK]
            eb = ep.tile([P, KVT_B, BLK], BF16, tag="exp", bufs=4, name="eb")
            for g in range(nkv // 4):
                # 4 scores tiles into one 2-bank psum -> one batched exp
                sc4 = ps2.tile([P, 4, BLK], F32, tag="sc4", bufs=2, name="sc4")
                for j in range(4):
                    kvt = g * 4 + j
                    nc.tensor.matmul(sc4[:, j, :],
                                     k_fm[:, kvh, kvt * P:(kvt + 1) * P], qs)
                nc.scalar.activation(eb[:, g * 4:(g + 1) * 4, :], sc4[:],
                                     AFT.Exp, scale=SCALE)
                if b == 0 or g == 1:
                    # block B kv tiles 0-3 are causally clean on every core
                    nc.vector.tensor_mul(
                        out=eb[:, g * 4:(g + 1) * 4, :],
                        in0=eb[:, g * 4:(g + 1) * 4, :],
                        in1=mask_sb[:, moff + g * 4:moff + (g + 1) * 4, :])
            return (h, b, nkv, eb)

        def emit_pv(st):
            h, b, nkv, eb = st
            kvh = h // REP
            aps = ps2.tile([P, BLK], F32, tag="attnps", bufs=2, name="aps")
            dps = ps2.tile([P, BLK], F32, tag="denps", bufs=2, name="dps")
            for kvt in range(nkv):
                nc.tensor.matmul(aps[:], v_tm[:, kvt, kvh * P:(kvh + 1) * P],
                                 eb[:, kvt, :],
                                 start=(kvt == 0), stop=(kvt == nkv - 1))
                nc.tensor.matmul(dps[:], ones_pp[:], eb[:, kvt, :],
                                 start=(kvt == 0), stop=(kvt == nkv - 1))
            rec = tp.tile([P, BLK], F32, tag="rec", bufs=3, name="rec")
            # ~18 correct bits -- plenty for a softmax denominator
            nc.vector.reciprocal_approx_fast(out=rec[:], in_=dps[:])
            # rec = SA/den, so attn_cat = SA*attn fits fp8 comfortably
            nc.vector.tensor_mul(out=attn_cat[:, h, b * BLK:(b + 1) * BLK],
                                 in0=aps[:], in1=rec[:])

        prev = None
        for h in range(NH):
            for b in range(2):
                st = emit_sc(h, b)
                if prev is not None:
                    emit_pv(prev)
                prev = st
        emit_pv(prev)

        exp_cm.__exit__(None, None, None)
        qkv_cm.__exit__(None, None, None)
        ps2_cm.__exit__(None, None, None)

        # ---- Phase 3: o_proj (fp8 DoubleRow) + residual + ln2 ----
        late_cm = tc.tile_pool(name="late", bufs=1, side="right")
        lp = late_cm.__enter__()
        ps3_cm = tc.tile_pool(name="ps3", bufs=1, space="PSUM")
        ps3 = ps3_cm.__enter__()

        h_res = lp.tile([P, KT, T], F32, tag="hres")
        mlp_in = lp.tile([P, KT, T], BF16, tag="mlpin")

        s2 = ps3.tile([P, 512], F32, tag="s2")
        for mb in range(4):
            wt = wp8.tile([P, KT, 512], FP8, tag="wchunk")
            nc.sync.dma_start(wt[:], d_wo[mb])
            for ms in range(4):
                mt = mb * 4 + ms
                ps = ps3.tile([P, T], F32, tag="mm", bufs=4)
                for k in range(0, KT, 2):
                    nc.tensor.matmul(ps[:], wt[:, k:k + 2, ms * P:(ms + 1) * P],
                                     attn_cat[:, k:k + 2, :],
                                     start=(k == 0), stop=(k == KT - 2),
                                     perf_mode=DR)
                xres = tp.tile([P, T], F32, tag="xres")
                nc.sync.dma_start(xres[:], d_xqres[:, mt, :])
                # h = ps * (1/(SA*s_wo)) + x   (fp8 dequant fused into the add)
                nc.vector.scalar_tensor_tensor(
                    out=h_res[:, mt, :], in0=ps[:], scalar=consts[:, 2:3],
                    in1=xres[:], op0=MUL, op1=ADD)
                sq2 = tp.tile([P, T], BF16, tag="sqq")
                nc.vector.tensor_mul(out=sq2[:], in0=h_res[:, mt, :],
                                     in1=h_res[:, mt, :])
                nc.tensor.matmul(s2[:], ones_pp[:], sq2[:],
                                 start=(mt == 0), stop=(mt == KT - 1))
        # d2 = 1/sqrt(s2*SA/H + eps); sqrt reads psum directly, scale/bias fused
        d2s = tp.tile([P, T], F32, tag="d2s")
        nc.scalar.activation(d2s[:], s2[:], AFT.Sqrt, bias=consts[:, 3:4],
                             scale=SA / H)
        d2scr = tp.tile([P, T], F32, tag="d2scr")
        nc.vector.reciprocal_approx_accurate(out=d2b[:], in_=d2s[:],
                                             scratch=d2scr[:])
        for k in range(KT):
            nc.vector.tensor_mul(out=mlp_in[:, k, :], in0=h_res[:, k, :], in1=d2b[:])

        attn_cm.__exit__(None, None, None)
        ps3_cm.__exit__(None, None, None)
        wp8_cm.__exit__(None, None, None)

        # ---- Phase 4: gate_up + SwiGLU ----
        mlp_cm = tc.tile_pool(name="mlp", bufs=1)
        mp = mlp_cm.__enter__()
        ps45_cm = tc.tile_pool(name="ps45", bufs=6, space="PSUM")
        ps45 = ps45_cm.__enter__()

        mid = mp.tile([P, IT, T], BF16, tag="mid")
        # chunk mb columns: [gate[mb*256:(mb+1)*256], up[mb*256:(mb+1)*256]]
        for mb in range(32):
            wt = wp.tile([P, KT, 512], BF16, tag="wchunk")
            nc.sync.dma_start(wt[:], d_wgu[mb])
            pss = []
            for ms in range(4):
                ps = ps45.tile([P, T], F32, tag="mm")
                for k in range(KT):
                    nc.tensor.matmul(ps[:], wt[:, k, ms * P:(ms + 1) * P],
                                     mlp_in[:, k, :],
                                     start=(k == 0), stop=(k == KT - 1))
                pss.append(ps)
            for j in range(2):
                # silu(g)*u = sigmoid(g)*g*u  (Silu table not in CoreSim)
                sg = tp.tile([P, T], F32, tag="silu")
                nc.scalar.activation(sg[:], pss[j][:], AFT.Sigmoid)
                t2 = tp.tile([P, T], F32, tag="silu2")
                nc.vector.tensor_mul(out=t2[:], in0=sg[:], in1=pss[j][:])
                nc.vector.tensor_mul(out=mid[:, 2 * mb + j, :], in0=t2[:],
                                     in1=pss[2 + j][:])

        # ---- Phase 5: down proj + residual ----
        for mt in range(KT):
            wt = wp.tile([P, IT, P], BF16, tag="wchunk")
            nc.sync.dma_start(wt[:], d_wd[mt])
            ps = ps45.tile([P, T], F32, tag="mm")
            for k in range(IT):
                nc.tensor.matmul(ps[:], wt[:, k, :], mid[:, k, :],
                                 start=(k == 0), stop=(k == IT - 1))
            yt = tp.tile([P, T], F32, tag="yt")
            nc.vector.tensor_add(out=yt[:], in0=ps[:], in1=h_res[:, mt, :])
            nc.sync.dma_start(d_y[:, mt, :], yt[:])

        mlp_cm.__exit__(None, None, None)
        ps45_cm.__exit__(None, None, None)
        late_cm.__exit__(None, None, None)
        tp_cm.__exit__(None, None, None)
        wp_cm.__exit__(None, None, None)
        glob_cm.__exit__(None, None, None)

    nc.compile()
    return nc


# ---------------- host-side preparation ----------------

def _perm(half):
    # q tokens sit at perm[Q0:Q0+T); block A's causal prefix fits in perm[0:512)
    if half == 0:
        return np.concatenate([np.arange(256, 512), np.arange(0, 256),
                               np.arange(768, 1024), np.arange(512, 768)])
    return np.arange(SKV)


def _pack_w(WT, mcol):
    # WT [K, M] -> [M//mcol, 128, K//128, mcol]; arr[mb,p,k,m] = WT[k*128+p, mb*mcol+m]
    K, M = WT.shape
    a = WT.reshape(K // P, P, M // mcol, mcol).transpose(2, 1, 0, 3)
    return np.ascontiguousarray(a)


def _q8(a, s):
    return np.clip(a * s, -240.0, 240.0).astype(F8)


def _prep_shared(inputs):
    w_ln1 = np.asarray(inputs["w_ln1"], np.float32)
    w_ln2 = np.asarray(inputs["w_ln2"], np.float32)
    w_q = np.asarray(inputs["w_q"], np.float32) * w_ln1[None, :]
    w_k = np.asarray(inputs["w_k"], np.float32) * w_ln1[None, :]
    w_v = np.asarray(inputs["w_v"], np.float32) * w_ln1[None, :]
    w_o = np.asarray(inputs["w_o"], np.float32)
    w_gu = np.asarray(inputs["w_gate_up"], np.float32) * w_ln2[None, :]
    w_d = np.asarray(inputs["w_down"], np.float32)

    x = np.asarray(inputs["hidden_states"], np.float32)
    s_x = 240.0 / max(float(np.abs(x).max()), 1e-30)
    wmax = max(float(np.abs(w_q).max()), float(np.abs(w_k).max()),
               float(np.abs(w_v).max()))
    s_w = 240.0 / max(wmax, 1e-30)
    s_wo = 240.0 / max(float(np.abs(w_o).max()), 1e-30)
    c_qk = 1.0 / (s_x * s_w)

    wq = _q8(_pack_w(w_q.T, 512), s_w)                  # [4,128,16,512]
    wk = _q8(_pack_w(w_k.T, 512)[0], s_w)               # [128,16,512]
    wv = _q8(_pack_w(w_v.T, 512)[0], s_w)
    wo = _q8(_pack_w(w_o.T, 512), s_wo)
    # gate/up interleave: chunk mb = [gate cols mb*256..], [up cols mb*256..]
    WT_gu = w_gu.T                                      # [H, 2I]
    cols = np.empty((32, 512), np.int64)
    for mb in range(32):
        cols[mb, :256] = np.arange(mb * 256, (mb + 1) * 256)
        cols[mb, 256:] = I + np.arange(mb * 256, (mb + 1) * 256)
    wgu = _pack_w(np.ascontiguousarray(WT_gu[:, cols.reshape(-1)]), 512).astype(BF)
    wd = _pack_w(w_d.T, 128).astype(BF)                 # [16,128,64,128]

    consts = np.empty((P, 4), np.float32)
    consts[:, 0] = SA / (H * s_x * s_x * c_qk * c_qk)
    consts[:, 1] = EPS / (c_qk * c_qk)
    consts[:, 2] = 1.0 / (SA * s_wo)
    consts[:, 3] = EPS

    sin_t = np.asarray(inputs["sin_table"], np.float32)   # [S, 64]
    cos_t = np.asarray(inputs["cos_table"], np.float32)

    def rope_tables(pos):
        C = np.empty((P, len(pos)), np.float32)
        Sg = np.empty((P, len(pos)), np.float32)
        c = cos_t[pos, :].T                      # [64, n]
        s = sin_t[pos, :].T
        C[0:64] = c
        C[64:128] = c
        Sg[0:64] = -s
        Sg[64:128] = s
        return C, Sg

    per_half = {}
    for half in range(2):
        perm = _perm(half)
        C, Sg = rope_tables(perm)
        qpos = perm[Q0:Q0 + T]
        m = np.zeros((P, NMSK, BLK), np.float32)
        for b in range(2):
            qpb = qpos[b * BLK:(b + 1) * BLK]
            nkv = KVT_A if b == 0 else KVT_B
            moff = 0 if b == 0 else KVT_A
            for kvt in range(nkv):
                kvp = perm[kvt * P:(kvt + 1) * P]
                m[:, moff + kvt, :] = (kvp[:, None] <= qpb[None, :])
        per_half[half] = dict(perm=perm, ckv=C, skv=Sg, mask=m.astype(BF))

    ones_pp = np.full((P, P), 1.0 / SA, BF)
    eye_pp = np.eye(P, dtype=np.float32)
    return dict(wq=wq, wk=wk, wv=wv, wo=wo, wgu=wgu, wd=wd, s_x=s_x,
                per_half=per_half, ones_pp=ones_pp, eye_pp=eye_pp,
                consts=consts)


def _core_in_map(shared, x, core):
    b, half = core // 2, core % 2
    ph = shared["per_half"][half]
    xT = x[b].T[:, ph["perm"]]                           # [H, SKV] permuted
    x_pack = np.ascontiguousarray(xT.reshape(KT, P, SKV).transpose(1, 0, 2))
    return {
        "x_kv": _q8(x_pack, shared["s_x"]),
        "x_qres": np.ascontiguousarray(x_pack[:, :, Q0:Q0 + T], np.float32),
        "ckv": ph["ckv"], "skv": ph["skv"], "mask": ph["mask"],
        "ones_pp": shared["ones_pp"], "eye_pp": shared["eye_pp"],
        "consts": shared["consts"],
        "wq": shared["wq"], "wk": shared["wk"], "wv": shared["wv"],
        "wo": shared["wo"], "wgu": shared["wgu"], "wd": shared["wd"],
    }


_NC = None


def kernel(**inputs):
    global _NC, LAST_RESULT
    if _NC is None:
        _NC = build_nc()
    nc = _NC

    shared = _prep_shared(inputs)
    x = np.asarray(inputs["hidden_states"], np.float32)    # [B,S,H]
    in_maps = [_core_in_map(shared, x, c) for c in range(N_CORES)]

    trace = bool(int(os.environ.get("BASS_TRACE", "0") or "0"))
    res = None
    for attempt in range(3):
        try:
            res = run_bass_kernel_spmd(nc, in_maps, core_ids=list(range(N_CORES)),
                                       trace=trace)
            break
        except Exception:
            # the axon terminal occasionally wedges transiently (LoadExecutable
            # failures); it recovers after a short idle
            if attempt == 2:
                raise
            import time
            time.sleep(90)
    LAST_RESULT = res

    out = np.empty((B, S, H), np.float32)
    for c in range(N_CORES):
        b, half = c // 2, c % 2
        qpos = _perm(half)[Q0:Q0 + T]
        y = res.results[c]["y"]                            # [128,16,512]
        out[b, qpos, :] = y.transpose(1, 0, 2).reshape(H, T).T
    return out


# revision 18
# speedup vs baseline: 1.0406x; 1.0406x over previous
"""Trainium2 Bass kernel for a dense decoder layer (RMSNorm -> GQA attn -> RMSNorm -> SwiGLU MLP).

Sharding: token-parallel across 8 cores (no collectives). Each core owns 512
query tokens of one batch (two causally balanced 256-token blocks) and computes
K/V for its batch's full 1024 tokens. Host permutes the batch's token columns
per core so q tokens always sit at columns [256:768) and block A's causal
prefix fits in kv columns [0:512) -- the compiled program is identical on all
cores. Activations are feature-major [feature_part, token_free] so matmuls
chain with no transposes; V is produced token-major by swapping matmul operand
roles.

Precision: Q/K/V/O projections run in fp8e4 (DoubleRow perf mode, 2 k-tiles
per matmul) with fp32 PSUM accumulation; x and those four weights are
quantized host-side with per-tensor absmax scales. The dequant constant
1/(s_x*s_w) is folded into the rmsnorm-1 rsqrt (imm/eps divided by c^2), so
d1b carries it into the rope tables and V's per-token scale for free. The
attention output is written as fp8 scaled by s_a=32 (exact in bf16), with
1/s_a folded into the all-ones stats/denominator matmul weight and the
residual add carrying 1/(s_a*s_wo). Scores/PV and the whole MLP stay bf16
(fp8 there busts the 2e-2 error budget). Runtime-dependent dequant scalars
ride in a tiny "consts" input tensor since immediates are compile-time.

Softmax skips max-subtraction (|scores| < ~6) and causality is a host-built
0/1 mask multiplied after exp. Partition-axis reductions use an all-[1/32]
[128,128] stationary matmul, which also broadcasts results across partitions.
"""

import os

import numpy as np
import ml_dtypes

import concourse.bass as bass  # noqa: F401
import concourse.mybir as mybir
import concourse.tile as tile
from concourse import bacc
from concourse.bass_utils import run_bass_kernel_spmd

# ---- problem shapes (hardcoded) ----
B, S, H = 4, 1024, 2048
NH, KVH, HD = 16, 4, 128
I = 8192
EPS = 1e-6

P = 128
KT = H // P            # 16 k-tiles over H
T = 512                # q tokens per core
SKV = 1024             # kv tokens per core (its batch's full sequence)
DV = KVH * HD          # 512
REP = NH // KVH
N_CORES = 8
BLK = 256              # q block size
KVT_A, KVT_B = 4, 8    # kv tiles processed for block A / block B
NMSK = KVT_A + KVT_B   # 12
IT = I // P            # 64 k-tiles over I
Q0 = 256               # q tokens live at columns [Q0 : Q0+T)
SCALE = 1.0 / float(np.sqrt(HD))
SA = 32.0              # attn fp8 scale; 1/32 is exact in bf16
N8 = 3                 # gate_up chunks (of 32) run in fp8; tail of the I dim
NBF = 32 - N8          # bf16 gate_up chunks
ITBF = IT - 2 * N8     # bf16 down-proj k-tiles
S_MI = 15.0            # mlp_in fp8 scale (|mlp_in| < 16 by rmsnorm bound)
S_MID = 5.0            # mid fp8 scale (|mid| < 48)

F32 = mybir.dt.float32
BF16 = mybir.dt.bfloat16
FP8 = mybir.dt.float8e4
MUL = mybir.AluOpType.mult
ADD = mybir.AluOpType.add
AFT = mybir.ActivationFunctionType
DR = mybir.MatmulPerfMode.DoubleRow

BF = ml_dtypes.bfloat16
F8 = ml_dtypes.float8_e4m3fn

LAST_RESULT = None  # BassKernelResults of the most recent run (for test harness)


def _install_ntff_hook():
    """The agent image's `antenv` lacks `axon_hooks`, so the boot shim's NTFF
    hook registration degrades silently and bass_utils crashes on import of
    antenv.axon_hooks when trace=True. Recreate the module and register the
    ctypes-based hook from trn_agent_boot."""
    import sys
    import types
    try:
        import antenv.axon_hooks  # noqa: F401
        return
    except ImportError:
        pass
    try:
        import antenv
    except ImportError:
        return
    mod = types.ModuleType("antenv.axon_hooks")
    _hook = [None]
    mod.set_axon_ntff_profile_hook = lambda h: _hook.__setitem__(0, h)
    mod.get_axon_ntff_profile_hook = lambda: _hook[0]
    sys.modules["antenv.axon_hooks"] = mod
    antenv.axon_hooks = mod
    try:
        from trn_agent_boot import trn_boot
        import ctypes
        so_path = "/opt/axon/libaxon_pjrt.so"
        lib = ctypes.CDLL(so_path)
        if hasattr(lib, "axon_start_nrt_profile"):
            mod.set_axon_ntff_profile_hook(
                trn_boot._ntff_profile_via_ctypes(so_path))
    except Exception:
        pass


_install_ntff_hook()


def build_nc():
    nc = bacc.Bacc(
        "TRN2",
        target_bir_lowering=False,
        debug=False,
        enable_asserts=False,
        num_devices=N_CORES,
    )

    # ---- DRAM I/O ----
    d_xkv = nc.dram_tensor("x_kv", [P, KT, SKV], FP8, kind="ExternalInput")
    d_xqres = nc.dram_tensor("x_qres", [P, KT, T], F32, kind="ExternalInput")
    d_ckv = nc.dram_tensor("ckv", [P, SKV], F32, kind="ExternalInput")
    d_skv = nc.dram_tensor("skv", [P, SKV], F32, kind="ExternalInput")
    d_consts = nc.dram_tensor("consts", [P, 8], F32, kind="ExternalInput")
    d_mask = nc.dram_tensor("mask", [P, NMSK, BLK], BF16, kind="ExternalInput")
    d_ones = nc.dram_tensor("ones_pp", [P, P], BF16, kind="ExternalInput")
    d_eye = nc.dram_tensor("eye_pp", [P, P], F32, kind="ExternalInput")
    d_wq = nc.dram_tensor("wq", [4, P, KT, 512], FP8, kind="ExternalInput")
    d_wk = nc.dram_tensor("wk", [P, KT, 512], FP8, kind="ExternalInput")
    d_wv = nc.dram_tensor("wv", [P, KT, 512], FP8, kind="ExternalInput")
    d_wo = nc.dram_tensor("wo", [4, P, KT, 512], FP8, kind="ExternalInput")
    d_wgu = nc.dram_tensor("wgu", [NBF, P, KT, 512], BF16, kind="ExternalInput")
    d_wgu8 = nc.dram_tensor("wgu8", [N8, P, KT, 512], FP8, kind="ExternalInput")
    d_wd = nc.dram_tensor("wd", [16, P, ITBF, P], BF16, kind="ExternalInput")
    d_wd8 = nc.dram_tensor("wd8", [16, P, 2 * N8, P], FP8, kind="ExternalInput")
    d_y = nc.dram_tensor("y", [P, KT, T], F32, kind="ExternalOutput")

    with tile.TileContext(nc) as tc:
        # long-lived pools
        glob_cm = tc.tile_pool(name="glob", bufs=1)
        gp = glob_cm.__enter__()
        wp_cm = tc.tile_pool(name="wstream", bufs=2)
        wp = wp_cm.__enter__()
        tp_cm = tc.tile_pool(name="tmp", bufs=2)
        tp = tp_cm.__enter__()
        wp8_cm = tc.tile_pool(name="wstream8", bufs=2)
        wp8 = wp8_cm.__enter__()
        # scratch that dies with phase 3 (freed before the MLP pool opens)
        tpE_cm = tc.tile_pool(name="tmpE", bufs=2)
        tpE = tpE_cm.__enter__()

        ones_pp = gp.tile([P, P], BF16, tag="ones")
        consts = gp.tile([P, 8], F32, tag="consts")
        d2b = gp.tile([P, T], F32, tag="d2b")

        # ---- Phase 0: RMSNorm1 stats (projections run on raw fp8 x; the
        # per-token scale d1 is applied post-matmul, so the PE never waits
        # for the stats chain) ----
        pA_cm = tc.tile_pool(name="ph01", bufs=1)
        pA = pA_cm.__enter__()
        ps01_cm = tc.tile_pool(name="ps01", bufs=1, space="PSUM")
        ps01 = ps01_cm.__enter__()

        ckv_sb = pA.tile([P, SKV], F32, tag="ckv")
        skv_sb = pA.tile([P, SKV], F32, tag="skv")
        x_bf = pA.tile([P, KT, SKV], FP8, tag="xbf")
        d1b = pA.tile([P, SKV], F32, tag="d1b")
        d1s = pA.tile([P, SKV], F32, tag="d1s")
        d1t = pA.tile([P, SKV // P], F32, tag="d1t")   # token-major d1

        s1a = ps01.tile([P, 512], F32, tag="s1a")
        s1b = ps01.tile([P, 512], F32, tag="s1b")
        # x chunk 0 gates the whole stats chain -- DMA it before everything else
        nc.sync.dma_start(x_bf[:, 0, :], d_xkv[:, 0, :])
        nc.sync.dma_start(ones_pp[:], d_ones[:])
        nc.sync.dma_start(consts[:], d_consts[:])
        for k in range(KT):
            if k > 0:
                nc.sync.dma_start(x_bf[:, k, :], d_xkv[:, k, :])
            sqv = tpE.tile([P, SKV], BF16, tag="sqkv")
            nc.vector.tensor_mul(out=sqv[:], in0=x_bf[:, k, :], in1=x_bf[:, k, :])
            nc.tensor.matmul(s1a[:], ones_pp[:], sqv[:, 0:512],
                             start=(k == 0), stop=(k == KT - 1))
            nc.tensor.matmul(s1b[:], ones_pp[:], sqv[:, 512:1024],
                             start=(k == 0), stop=(k == KT - 1))
        # tables DMA'd after the x stream so they don't delay the first matmuls
        nc.sync.dma_start(ckv_sb[:], d_ckv[:])
        nc.sync.dma_start(skv_sb[:], d_skv[:])
        # d1b = c_qk/sqrt(s/H + eps): the fp8 dequant constant is folded into
        # the sqrt scale/bias (divided by c^2), so no extra op is needed.
        # consts[:,0] = SA/(H*s_x^2*c^2), consts[:,1] = EPS/c^2.
        for half, ps in ((0, s1a), (1, s1b)):
            sl = slice(half * 512, (half + 1) * 512)
            nc.scalar.activation(d1s[:, sl], ps[:], AFT.Sqrt,
                                 bias=consts[:, 1:2], scale=consts[:, 0:1])
            scr = tpE.tile([P, 512], F32, tag="d1scr")
            nc.vector.reciprocal_approx_accurate(out=d1b[:, sl], in_=d1s[:, sl],
                                                 scratch=scr[:])
        ps01_cm.__exit__(None, None, None)
        # fold the per-token d1*c into the rope tables (saves one DVE mul per tile)
        nc.vector.tensor_mul(out=ckv_sb[:], in0=ckv_sb[:], in1=d1b[:])
        nc.vector.tensor_mul(out=skv_sb[:], in0=skv_sb[:], in1=d1b[:])
        eye_sb = pA.tile([P, P], F32, tag="eye")
        nc.sync.dma_start(eye_sb[:], d_eye[:])

        psmm1_cm = tc.tile_pool(name="psmm1", bufs=6, space="PSUM")
        psmm1 = psmm1_cm.__enter__()

        # ---- Phase 1: Q/K/V projections (+rope), fp8 DoubleRow ----
        qkv_cm = tc.tile_pool(name="qkv", bufs=1, side="right")
        qp_ = qkv_cm.__enter__()
        q_fm = qp_.tile([P, NH, T], BF16, tag="qfm")
        k_fm = qp_.tile([P, KVH, SKV], BF16, tag="kfm")
        v_tm = qp_.tile([P, SKV // P, DV], BF16, tag="vtm")

        def rope_out(ps, cos_t, sin_t, out_ap, n):
            # out = raw*cos' + swap_halves(raw)*sin'  (d1, dequant + sin sign
            # pre-folded into the tables). The cos term reads PSUM directly so
            # the psum tile is released after [copy, mul] and never waits for
            # the swap DMA round-trip.
            raw = tpE.tile([P, n], BF16, tag="rope_raw")
            nc.scalar.copy(raw[:], ps[:])
            rawc = tpE.tile([P, n], F32, tag="rope_rc")
            nc.vector.tensor_mul(out=rawc[:], in0=raw[:], in1=cos_t)
            sw = tpE.tile([P, n], BF16, tag="rope_sw")
            nc.sync.dma_start(sw[0:64, :], raw[64:128, :])
            nc.sync.dma_start(sw[64:128, :], raw[0:64, :])
            nc.vector.tensor_mul(out=sw[:], in0=sw[:], in1=sin_t)
            nc.vector.tensor_add(out=out_ap, in0=rawc[:], in1=sw[:])

        # Q: 16 heads; q tokens are x_bf columns [Q0 : Q0+T)
        for mb in range(4):
            wt = wp8.tile([P, KT, 512], FP8, tag="wchunk")
            nc.sync.dma_start(wt[:], d_wq[mb])
            for ms in range(4):
                h = mb * 4 + ms
                ps = psmm1.tile([P, T], F32, tag="mm")
                for k in range(0, KT, 2):
                    nc.tensor.matmul(ps[:], wt[:, k:k + 2, ms * P:(ms + 1) * P],
                                     x_bf[:, k:k + 2, Q0:Q0 + T],
                                     start=(k == 0), stop=(k == KT - 2),
                                     perf_mode=DR)
                rope_out(ps, ckv_sb[:, Q0:Q0 + T], skv_sb[:, Q0:Q0 + T],
                         q_fm[:, h, :], T)

        # K: 4 kv heads x 2 halves of the kv sequence
        wtk = wp8.tile([P, KT, 512], FP8, tag="wchunk")
        nc.sync.dma_start(wtk[:], d_wk[:])
        for kvh in range(KVH):
            for half in range(2):
                ps = psmm1.tile([P, T], F32, tag="mm")
                for k in range(0, KT, 2):
                    nc.tensor.matmul(ps[:], wtk[:, k:k + 2, kvh * P:(kvh + 1) * P],
                                     x_bf[:, k:k + 2, half * 512:(half + 1) * 512],
                                     start=(k == 0), stop=(k == KT - 2),
                                     perf_mode=DR)
                rope_out(ps, ckv_sb[:, half * 512:(half + 1) * 512],
                         skv_sb[:, half * 512:(half + 1) * 512],
                         k_fm[:, kvh, half * 512:(half + 1) * 512], 512)

        # token-major d1 (for scaling V rows): PE-transpose each 128-col block
        # of the (row-broadcast) d1b and keep one column. Runs here (not in
        # phase 0) so the Q/K matmuls above aren't head-of-line blocked on the
        # d1 chain.
        for kvt in range(SKV // P):
            tps = psmm1.tile([P, P], F32, tag="tr", bufs=2)
            nc.tensor.transpose(tps[:], d1b[:, kvt * P:(kvt + 1) * P], eye_sb[:])
            nc.scalar.copy(d1t[:, kvt:kvt + 1], tps[:, 0:1])

        # V: token-major directly (lhsT = activations, rhs = weights)
        wtv = wp8.tile([P, KT, 512], FP8, tag="wchunk")
        nc.sync.dma_start(wtv[:], d_wv[:])
        for kvt in range(SKV // P):
            ps = psmm1.tile([P, DV], F32, tag="mm")
            for k in range(0, KT, 2):
                nc.tensor.matmul(ps[:], x_bf[:, k:k + 2, kvt * P:(kvt + 1) * P],
                                 wtv[:, k:k + 2, :],
                                 start=(k == 0), stop=(k == KT - 2),
                                 perf_mode=DR)
            # rows are tokens: apply per-token d1*c as a per-partition ACT scale
            nc.scalar.mul(v_tm[:, kvt, :], ps[:], d1t[:, kvt:kvt + 1])

        pA_cm.__exit__(None, None, None)
        psmm1_cm.__exit__(None, None, None)

        # ---- Phase 2: attention ----
        attn_cm = tc.tile_pool(name="attn", bufs=1)
        ap_ = attn_cm.__enter__()
        exp_cm = tc.tile_pool(name="exp", bufs=2)
        ep = exp_cm.__enter__()
        ps2_cm = tc.tile_pool(name="ps2", bufs=1, space="PSUM")
        ps2 = ps2_cm.__enter__()

        mask_sb = ap_.tile([P, NMSK, BLK], BF16, tag="mask")
        nc.sync.dma_start(mask_sb[:], d_mask[:])
        attn_cat = ap_.tile([P, NH, T], FP8, tag="attncat")

        # software pipeline: scores/exp of chain i+1 are emitted before the
        # PV/denominator matmuls of chain i, so the PE never head-of-line
        # blocks on the ACT exp latency
        def emit_sc(h, b):
            kvh = h // REP
            nkv = KVT_A if b == 0 else KVT_B
            moff = 0 if b == 0 else KVT_A
            qs = q_fm[:, h, b * BLK:(b + 1) * BLK]
            eb = ep.tile([P, KVT_B, BLK], BF16, tag="exp", bufs=4, name="eb")
            for g in range(nkv // 4):
                # 4 scores tiles into one 2-bank psum -> one batched exp
                sc4 = ps2.tile([P, 4, BLK], F32, tag="sc4", bufs=2, name="sc4")
                for j in range(4):
                    kvt = g * 4 + j
                    nc.tensor.matmul(sc4[:, j, :],
                                     k_fm[:, kvh, kvt * P:(kvt + 1) * P], qs)
                nc.scalar.activation(eb[:, g * 4:(g + 1) * 4, :], sc4[:],
                                     AFT.Exp, scale=SCALE)
                if b == 0 or g == 1:
                    # block B kv tiles 0-3 are causally clean on every core
                    nc.vector.tensor_mul(
                        out=eb[:, g * 4:(g + 1) * 4, :],
                        in0=eb[:, g * 4:(g + 1) * 4, :],
                        in1=mask_sb[:, moff + g * 4:moff + (g + 1) * 4, :])
            return (h, b, nkv, eb)

        def emit_pv(st):
            h, b, nkv, eb = st
            kvh = h // REP
            aps = ps2.tile([P, BLK], F32, tag="attnps", bufs=2, name="aps")
            dps = ps2.tile([P, BLK], F32, tag="denps", bufs=2, name="dps")
            for kvt in range(nkv):
                nc.tensor.matmul(aps[:], v_tm[:, kvt, kvh * P:(kvh + 1) * P],
                                 eb[:, kvt, :],
                                 start=(kvt == 0), stop=(kvt == nkv - 1))
                nc.tensor.matmul(dps[:], ones_pp[:], eb[:, kvt, :],
                                 start=(kvt == 0), stop=(kvt == nkv - 1))
            rec = tpE.tile([P, BLK], F32, tag="rec", bufs=3, name="rec")
            # ~18 correct bits -- plenty for a softmax denominator
            nc.vector.reciprocal_approx_fast(out=rec[:], in_=dps[:])
            # rec = SA/den, so attn_cat = SA*attn fits fp8 comfortably
            nc.vector.tensor_mul(out=attn_cat[:, h, b * BLK:(b + 1) * BLK],
                                 in0=aps[:], in1=rec[:])

        prev = None
        for h in range(NH):
            for b in range(2):
                st = emit_sc(h, b)
                if prev is not None:
                    emit_pv(prev)
                prev = st
        emit_pv(prev)

        exp_cm.__exit__(None, None, None)
        qkv_cm.__exit__(None, None, None)
        ps2_cm.__exit__(None, None, None)

        # ---- Phase 3: o_proj (fp8 DoubleRow) + residual + ln2 ----
        late_cm = tc.tile_pool(name="late", bufs=1, side="right")
        lp = late_cm.__enter__()
        ps3_cm = tc.tile_pool(name="ps3", bufs=1, space="PSUM")
        ps3 = ps3_cm.__enter__()

        h_res = lp.tile([P, KT, T], F32, tag="hres")
        mlp_in = lp.tile([P, KT, T], BF16, tag="mlpin")
        mlp_in8 = lp.tile([P, KT, T], FP8, tag="mlpin8")
        d2b8 = lp.tile([P, T], F32, tag="d2b8")

        s2 = ps3.tile([P, 512], F32, tag="s2")
        for mb in range(4):
            wt = wp8.tile([P, KT, 512], FP8, tag="wchunk")
            nc.sync.dma_start(wt[:], d_wo[mb])
            for ms in range(4):
                mt = mb * 4 + ms
                ps = ps3.tile([P, T], F32, tag="mm", bufs=4)
                for k in range(0, KT, 2):
                    nc.tensor.matmul(ps[:], wt[:, k:k + 2, ms * P:(ms + 1) * P],
                                     attn_cat[:, k:k + 2, :],
                                     start=(k == 0), stop=(k == KT - 2),
                                     perf_mode=DR)
                xres = tpE.tile([P, T], F32, tag="xres")
                nc.sync.dma_start(xres[:], d_xqres[:, mt, :])
                # h = ps * (1/(SA*s_wo)) + x   (fp8 dequant fused into the add)
                nc.vector.scalar_tensor_tensor(
                    out=h_res[:, mt, :], in0=ps[:], scalar=consts[:, 2:3],
                    in1=xres[:], op0=MUL, op1=ADD)
                sq2 = tpE.tile([P, T], BF16, tag="sqq")
                nc.vector.tensor_mul(out=sq2[:], in0=h_res[:, mt, :],
                                     in1=h_res[:, mt, :])
                nc.tensor.matmul(s2[:], ones_pp[:], sq2[:],
                                 start=(mt == 0), stop=(mt == KT - 1))
        # d2 = 1/sqrt(s2*SA/H + eps); sqrt reads psum directly, scale/bias fused
        d2s = tpE.tile([P, T], F32, tag="d2s")
        nc.scalar.activation(d2s[:], s2[:], AFT.Sqrt, bias=consts[:, 3:4],
                             scale=SA / H)
        d2scr = tpE.tile([P, T], F32, tag="d2scr")
        nc.vector.reciprocal_approx_accurate(out=d2b[:], in_=d2s[:],
                                             scratch=d2scr[:])
        for k in range(KT):
            nc.vector.tensor_mul(out=mlp_in[:, k, :], in0=h_res[:, k, :], in1=d2b[:])
        # fp8 copy of mlp_in (scaled by S_MI) for the fp8 gate_up chunks;
        # emitted after the bf16 tiles so the bf16 chunks start first
        nc.scalar.mul(d2b8[:], d2b[:], consts[:, 6:7])
        for k in range(KT):
            nc.vector.tensor_mul(out=mlp_in8[:, k, :], in0=h_res[:, k, :], in1=d2b8[:])

        attn_cm.__exit__(None, None, None)
        ps3_cm.__exit__(None, None, None)
        tpE_cm.__exit__(None, None, None)
        wp8_cm.__exit__(None, None, None)

        # ---- Phase 4: gate_up + SwiGLU ----
        mlp_cm = tc.tile_pool(name="mlp", bufs=1)
        mp = mlp_cm.__enter__()
        ps45_cm = tc.tile_pool(name="ps45", bufs=6, space="PSUM")
        ps45 = ps45_cm.__enter__()

        mid = mp.tile([P, IT, T], BF16, tag="mid")
        mid8 = mp.tile([P, 2 * N8, T], FP8, tag="mid8")
        # chunk mb columns: [gate[mb*256:(mb+1)*256], up[mb*256:(mb+1)*256]]
        for mb in range(32):
            f8 = mb >= NBF
            if f8:
                wt = mp.tile([P, KT, 512], FP8, tag="wchunk8", bufs=2)
                nc.sync.dma_start(wt[:], d_wgu8[mb - NBF])
            else:
                wt = wp.tile([P, KT, 512], BF16, tag="wchunk")
                nc.sync.dma_start(wt[:], d_wgu[mb])
            pss = []
            for ms in range(4):
                ps = ps45.tile([P, T], F32, tag="mm")
                if f8:
                    for k in range(0, KT, 2):
                        nc.tensor.matmul(ps[:], wt[:, k:k + 2, ms * P:(ms + 1) * P],
                                         mlp_in8[:, k:k + 2, :],
                                         start=(k == 0), stop=(k == KT - 2),
                                         perf_mode=DR)
                else:
                    for k in range(KT):
                        nc.tensor.matmul(ps[:], wt[:, k, ms * P:(ms + 1) * P],
                                         mlp_in[:, k, :],
                                         start=(k == 0), stop=(k == KT - 1))
                pss.append(ps)
            for j in range(2):
                # silu(g)*u = sigmoid(g)*g*u  (Silu table not in CoreSim)
                sg = tp.tile([P, T], F32, tag="silu")
                if f8:
                    # psum carries s_mi*s_wgu8; dequant c1 folded into each op
                    nc.scalar.activation(sg[:], pss[j][:], AFT.Sigmoid,
                                         scale=consts[:, 4:5])
                    t2 = tp.tile([P, T], F32, tag="silu2")
                    nc.vector.scalar_tensor_tensor(
                        out=t2[:], in0=pss[j][:], scalar=consts[:, 4:5],
                        in1=sg[:], op0=MUL, op1=MUL)
                    nc.vector.scalar_tensor_tensor(
                        out=mid8[:, 2 * (mb - NBF) + j, :], in0=pss[2 + j][:],
                        scalar=consts[:, 5:6], in1=t2[:], op0=MUL, op1=MUL)
                else:
                    nc.scalar.activation(sg[:], pss[j][:], AFT.Sigmoid)
                    t2 = tp.tile([P, T], F32, tag="silu2")
                    nc.vector.tensor_mul(out=t2[:], in0=sg[:], in1=pss[j][:])
                    nc.vector.tensor_mul(out=mid[:, 2 * mb + j, :], in0=t2[:],
                                         in1=pss[2 + j][:])

        # ---- Phase 5: down proj + residual ----
        for mt in range(KT):
            wt = wp.tile([P, ITBF, P], BF16, tag="wchunk")
            nc.sync.dma_start(wt[:], d_wd[mt])
            wt8 = mp.tile([P, 2 * N8, P], FP8, tag="wd8chunk", bufs=2)
            nc.sync.dma_start(wt8[:], d_wd8[mt])
            ps = ps45.tile([P, T], F32, tag="mm")
            for k in range(ITBF):
                nc.tensor.matmul(ps[:], wt[:, k, :], mid[:, k, :],
                                 start=(k == 0), stop=(k == ITBF - 1))
            ps8 = ps45.tile([P, T], F32, tag="mm")
            for k in range(0, 2 * N8, 2):
                nc.tensor.matmul(ps8[:], wt8[:, k:k + 2, :],
                                 mid8[:, k:k + 2, :],
                                 start=(k == 0), stop=(k == 2 * N8 - 2),
                                 perf_mode=DR)
            yt = tp.tile([P, T], F32, tag="yt")
            nc.vector.tensor_add(out=yt[:], in0=ps[:], in1=h_res[:, mt, :])
            yt2 = tp.tile([P, T], F32, tag="yt2")
            nc.vector.scalar_tensor_tensor(
                out=yt2[:], in0=ps8[:], scalar=consts[:, 7:8], in1=yt[:],
                op0=MUL, op1=ADD)
            nc.sync.dma_start(d_y[:, mt, :], yt2[:])

        mlp_cm.__exit__(None, None, None)
        ps45_cm.__exit__(None, None, None)
        late_cm.__exit__(None, None, None)
        tp_cm.__exit__(None, None, None)
        wp_cm.__exit__(None, None, None)
        glob_cm.__exit__(None, None, None)

    nc.compile()
    return nc


# ---------------- host-side preparation ----------------

def _perm(half):
    # q tokens sit at perm[Q0:Q0+T); block A's causal prefix fits in perm[0:512)
    if half == 0:
        return np.concatenate([np.arange(256, 512), np.arange(0, 256),
                               np.arange(768, 1024), np.arange(512, 768)])
    return np.arange(SKV)


def _pack_w(WT, mcol):
    # WT [K, M] -> [M//mcol, 128, K//128, mcol]; arr[mb,p,k,m] = WT[k*128+p, mb*mcol+m]
    K, M = WT.shape
    a = WT.reshape(K // P, P, M // mcol, mcol).transpose(2, 1, 0, 3)
    return np.ascontiguousarray(a)


def _q8(a, s):
    return np.clip(a * s, -240.0, 240.0).astype(F8)


def _prep_shared(inputs):
    w_ln1 = np.asarray(inputs["w_ln1"], np.float32)
    w_ln2 = np.asarray(inputs["w_ln2"], np.float32)
    w_q = np.asarray(inputs["w_q"], np.float32) * w_ln1[None, :]
    w_k = np.asarray(inputs["w_k"], np.float32) * w_ln1[None, :]
    w_v = np.asarray(inputs["w_v"], np.float32) * w_ln1[None, :]
    w_o = np.asarray(inputs["w_o"], np.float32)
    w_gu = np.asarray(inputs["w_gate_up"], np.float32) * w_ln2[None, :]
    w_d = np.asarray(inputs["w_down"], np.float32)

    x = np.asarray(inputs["hidden_states"], np.float32)
    s_x = 240.0 / max(float(np.abs(x).max()), 1e-30)
    wmax = max(float(np.abs(w_q).max()), float(np.abs(w_k).max()),
               float(np.abs(w_v).max()))
    s_w = 240.0 / max(wmax, 1e-30)
    s_wo = 240.0 / max(float(np.abs(w_o).max()), 1e-30)
    c_qk = 1.0 / (s_x * s_w)

    wq = _q8(_pack_w(w_q.T, 512), s_w)                  # [4,128,16,512]
    wk = _q8(_pack_w(w_k.T, 512)[0], s_w)               # [128,16,512]
    wv = _q8(_pack_w(w_v.T, 512)[0], s_w)
    wo = _q8(_pack_w(w_o.T, 512), s_wo)
    # gate/up interleave: chunk mb = [gate cols mb*256..], [up cols mb*256..]
    WT_gu = w_gu.T                                      # [H, 2I]
    cols = np.empty((32, 512), np.int64)
    for mb in range(32):
        cols[mb, :256] = np.arange(mb * 256, (mb + 1) * 256)
        cols[mb, 256:] = I + np.arange(mb * 256, (mb + 1) * 256)
    wgu_all = _pack_w(np.ascontiguousarray(WT_gu[:, cols.reshape(-1)]), 512)
    wgu = wgu_all[:NBF].astype(BF)
    wgu8 = _q8(wgu_all[NBF:], 240.0 / max(float(np.abs(w_gu).max()), 1e-30))
    wd_all = _pack_w(w_d.T, 128)                        # [16,128,64,128]
    wd = np.ascontiguousarray(wd_all[:, :, :ITBF, :]).astype(BF)
    wd8 = _q8(np.ascontiguousarray(wd_all[:, :, ITBF:, :]),
              240.0 / max(float(np.abs(w_d).max()), 1e-30))

    s_wgu8 = 240.0 / max(float(np.abs(w_gu).max()), 1e-30)
    s_wd8 = 240.0 / max(float(np.abs(w_d).max()), 1e-30)
    consts = np.empty((P, 8), np.float32)
    consts[:, 0] = SA / (H * s_x * s_x * c_qk * c_qk)
    consts[:, 1] = EPS / (c_qk * c_qk)
    consts[:, 2] = 1.0 / (SA * s_wo)
    consts[:, 3] = EPS
    consts[:, 4] = 1.0 / (S_MI * s_wgu8)
    consts[:, 5] = S_MID / (S_MI * s_wgu8)
    consts[:, 6] = S_MI
    consts[:, 7] = 1.0 / (S_MID * s_wd8)

    sin_t = np.asarray(inputs["sin_table"], np.float32)   # [S, 64]
    cos_t = np.asarray(inputs["cos_table"], np.float32)

    def rope_tables(pos):
        C = np.empty((P, len(pos)), np.float32)
        Sg = np.empty((P, len(pos)), np.float32)
        c = cos_t[pos, :].T                      # [64, n]
        s = sin_t[pos, :].T
        C[0:64] = c
        C[64:128] = c
        Sg[0:64] = -s
        Sg[64:128] = s
        return C, Sg

    per_half = {}
    for half in range(2):
        perm = _perm(half)
        C, Sg = rope_tables(perm)
        qpos = perm[Q0:Q0 + T]
        m = np.zeros((P, NMSK, BLK), np.float32)
        for b in range(2):
            qpb = qpos[b * BLK:(b + 1) * BLK]
            nkv = KVT_A if b == 0 else KVT_B
            moff = 0 if b == 0 else KVT_A
            for kvt in range(nkv):
                kvp = perm[kvt * P:(kvt + 1) * P]
                m[:, moff + kvt, :] = (kvp[:, None] <= qpb[None, :])
        per_half[half] = dict(perm=perm, ckv=C, skv=Sg, mask=m.astype(BF))

    ones_pp = np.full((P, P), 1.0 / SA, BF)
    eye_pp = np.eye(P, dtype=np.float32)
    return dict(wq=wq, wk=wk, wv=wv, wo=wo, wgu=wgu, wgu8=wgu8, wd=wd,
                wd8=wd8, s_x=s_x, per_half=per_half, ones_pp=ones_pp,
                eye_pp=eye_pp, consts=consts)


def _core_in_map(shared, x, core):
    b, half = core // 2, core % 2
    ph = shared["per_half"][half]
    xT = x[b].T[:, ph["perm"]]                           # [H, SKV] permuted
    x_pack = np.ascontiguousarray(xT.reshape(KT, P, SKV).transpose(1, 0, 2))
    return {
        "x_kv": _q8(x_pack, shared["s_x"]),
        "x_qres": np.ascontiguousarray(x_pack[:, :, Q0:Q0 + T], np.float32),
        "ckv": ph["ckv"], "skv": ph["skv"], "mask": ph["mask"],
        "ones_pp": shared["ones_pp"], "eye_pp": shared["eye_pp"],
        "consts": shared["consts"],
        "wq": shared["wq"], "wk": shared["wk"], "wv": shared["wv"],
        "wo": shared["wo"], "wgu": shared["wgu"], "wgu8": shared["wgu8"],
        "wd": shared["wd"], "wd8": shared["wd8"],
    }


_NC = None


def kernel(**inputs):
    global _NC, LAST_RESULT
    if _NC is None:
        _NC = build_nc()
    nc = _NC

    shared = _prep_shared(inputs)
    x = np.asarray(inputs["hidden_states"], np.float32)    # [B,S,H]
    in_maps = [_core_in_map(shared, x, c) for c in range(N_CORES)]

    trace = bool(int(os.environ.get("BASS_TRACE", "0") or "0"))
    res = None
    for attempt in range(3):
        try:
            res = run_bass_kernel_spmd(nc, in_maps, core_ids=list(range(N_CORES)),
                                       trace=trace)
            break
        except Exception:
            # the axon terminal occasionally wedges transiently (LoadExecutable
            # failures); it recovers after a short idle
            if attempt == 2:
                raise
            import time
            time.sleep(90)
    LAST_RESULT = res

    out = np.empty((B, S, H), np.float32)
    for c in range(N_CORES):
        b, half = c // 2, c % 2
        qpos = _perm(half)[Q0:Q0 + T]
        y = res.results[c]["y"]                            # [128,16,512]
        out[b, qpos, :] = y.transpose(1, 0, 2).reshape(H, T).T
    return out


# revision 21
# speedup vs baseline: 1.0686x; 1.0269x over previous
"""Trainium2 Bass kernel for a dense decoder layer (RMSNorm -> GQA attn -> RMSNorm -> SwiGLU MLP).

Sharding: token-parallel across 8 cores (no collectives). Each core owns 512
query tokens of one batch (two causally balanced 256-token blocks) and computes
K/V for its batch's full 1024 tokens. Host permutes the batch's token columns
per core so q tokens always sit at columns [256:768) and block A's causal
prefix fits in kv columns [0:512) -- the compiled program is identical on all
cores. Activations are feature-major [feature_part, token_free] so matmuls
chain with no transposes; V is produced token-major by swapping matmul operand
roles.

Precision: Q/K/V/O projections run in fp8e4 (DoubleRow perf mode, 2 k-tiles
per matmul) with fp32 PSUM accumulation; x and those four weights are
quantized host-side with per-tensor absmax scales. The dequant constant
1/(s_x*s_w) is folded into the rmsnorm-1 rsqrt (imm/eps divided by c^2), so
d1b carries it into the rope tables and V's per-token scale for free. The
attention output is written as fp8 scaled by s_a=32 (exact in bf16), with
1/s_a folded into the all-ones stats/denominator matmul weight and the
residual add carrying 1/(s_a*s_wo). Scores/PV and the whole MLP stay bf16
(fp8 there busts the 2e-2 error budget). Runtime-dependent dequant scalars
ride in a tiny "consts" input tensor since immediates are compile-time.

Softmax skips max-subtraction (|scores| < ~6) and causality is a host-built
0/1 mask multiplied after exp. Partition-axis reductions use an all-[1/32]
[128,128] stationary matmul, which also broadcasts results across partitions.
"""

import os

import numpy as np
import ml_dtypes

import concourse.bass as bass  # noqa: F401
import concourse.mybir as mybir
import concourse.tile as tile
from concourse import bacc
from concourse.bass_utils import run_bass_kernel_spmd

# ---- problem shapes (hardcoded) ----
B, S, H = 4, 1024, 2048
NH, KVH, HD = 16, 4, 128
I = 8192
EPS = 1e-6

P = 128
KT = H // P            # 16 k-tiles over H
T = 512                # q tokens per core
SKV = 1024             # kv tokens per core (its batch's full sequence)
DV = KVH * HD          # 512
REP = NH // KVH
N_CORES = 8
BLK = 256              # q block size
KVT_A, KVT_B = 4, 8    # kv tiles processed for block A / block B
NMSK = KVT_A + KVT_B   # 12
IT = I // P            # 64 k-tiles over I
Q0 = 256               # q tokens live at columns [Q0 : Q0+T)
SCALE = 1.0 / float(np.sqrt(HD))
SA = 32.0              # attn fp8 scale; 1/32 is exact in bf16
N8 = 4                 # gate_up chunks (of 32) run in fp8; tail of the I dim
NBF = 32 - N8          # bf16 gate_up chunks
ITBF = IT - 2 * N8     # bf16 down-proj k-tiles
S_MI = 15.0            # mlp_in fp8 scale (|mlp_in| < 16 by rmsnorm bound)
S_MID = 5.0            # mid fp8 scale (|mid| < 48)

F32 = mybir.dt.float32
BF16 = mybir.dt.bfloat16
FP8 = mybir.dt.float8e4
MUL = mybir.AluOpType.mult
ADD = mybir.AluOpType.add
AFT = mybir.ActivationFunctionType
DR = mybir.MatmulPerfMode.DoubleRow

BF = ml_dtypes.bfloat16
F8 = ml_dtypes.float8_e4m3fn

LAST_RESULT = None  # BassKernelResults of the most recent run (for test harness)


def _install_ntff_hook():
    """The agent image's `antenv` lacks `axon_hooks`, so the boot shim's NTFF
    hook registration degrades silently and bass_utils crashes on import of
    antenv.axon_hooks when trace=True. Recreate the module and register the
    ctypes-based hook from trn_agent_boot."""
    import sys
    import types
    try:
        import antenv.axon_hooks  # noqa: F401
        return
    except ImportError:
        pass
    try:
        import antenv
    except ImportError:
        return
    mod = types.ModuleType("antenv.axon_hooks")
    _hook = [None]
    mod.set_axon_ntff_profile_hook = lambda h: _hook.__setitem__(0, h)
    mod.get_axon_ntff_profile_hook = lambda: _hook[0]
    sys.modules["antenv.axon_hooks"] = mod
    antenv.axon_hooks = mod
    try:
        from trn_agent_boot import trn_boot
        import ctypes
        so_path = "/opt/axon/libaxon_pjrt.so"
        lib = ctypes.CDLL(so_path)
        if hasattr(lib, "axon_start_nrt_profile"):
            mod.set_axon_ntff_profile_hook(
                trn_boot._ntff_profile_via_ctypes(so_path))
    except Exception:
        pass


_install_ntff_hook()


def build_nc():
    nc = bacc.Bacc(
        "TRN2",
        target_bir_lowering=False,
        debug=False,
        enable_asserts=False,
        num_devices=N_CORES,
    )

    # ---- DRAM I/O ----
    d_xkv = nc.dram_tensor("x_kv", [P, KT, SKV], FP8, kind="ExternalInput")
    d_xqres = nc.dram_tensor("x_qres", [P, KT, T], F32, kind="ExternalInput")
    d_ckv = nc.dram_tensor("ckv", [P, SKV], F32, kind="ExternalInput")
    d_skv = nc.dram_tensor("skv", [P, SKV], F32, kind="ExternalInput")
    d_consts = nc.dram_tensor("consts", [P, 8], F32, kind="ExternalInput")
    d_mask = nc.dram_tensor("mask", [P, NMSK, BLK], BF16, kind="ExternalInput")
    d_ones = nc.dram_tensor("ones_pp", [P, P], BF16, kind="ExternalInput")
    d_eye = nc.dram_tensor("eye_pp", [P, P], F32, kind="ExternalInput")
    d_wq = nc.dram_tensor("wq", [4, P, KT, 512], FP8, kind="ExternalInput")
    d_wk = nc.dram_tensor("wk", [P, KT, 512], FP8, kind="ExternalInput")
    d_wv = nc.dram_tensor("wv", [P, KT, 512], FP8, kind="ExternalInput")
    d_wo = nc.dram_tensor("wo", [4, P, KT, 512], FP8, kind="ExternalInput")
    d_wgu = nc.dram_tensor("wgu", [NBF, P, KT, 512], BF16, kind="ExternalInput")
    d_wgu8 = nc.dram_tensor("wgu8", [N8, P, KT, 512], FP8, kind="ExternalInput")
    d_wd = nc.dram_tensor("wd", [16, P, ITBF, P], BF16, kind="ExternalInput")
    d_wd8 = nc.dram_tensor("wd8", [16, P, 2 * N8, P], FP8, kind="ExternalInput")
    d_y = nc.dram_tensor("y", [P, KT, T], F32, kind="ExternalOutput")

    with tile.TileContext(nc) as tc:
        # long-lived pools
        glob_cm = tc.tile_pool(name="glob", bufs=1)
        gp = glob_cm.__enter__()
        wp_cm = tc.tile_pool(name="wstream", bufs=2)
        wp = wp_cm.__enter__()
        tp_cm = tc.tile_pool(name="tmp", bufs=2)
        tp = tp_cm.__enter__()
        wp8_cm = tc.tile_pool(name="wstream8", bufs=2)
        wp8 = wp8_cm.__enter__()
        # scratch that dies with phase 3 (freed before the MLP pool opens)
        tpE_cm = tc.tile_pool(name="tmpE", bufs=2)
        tpE = tpE_cm.__enter__()

        ones_pp = gp.tile([P, P], BF16, tag="ones")
        consts = gp.tile([P, 8], F32, tag="consts")
        d2b = gp.tile([P, T], F32, tag="d2b")
        mask_sb = gp.tile([P, NMSK, BLK], BF16, tag="mask")

        # ---- Phase 0: RMSNorm1 stats (projections run on raw fp8 x; the
        # per-token scale d1 is applied post-matmul, so the PE never waits
        # for the stats chain) ----
        pA_cm = tc.tile_pool(name="ph01", bufs=1)
        pA = pA_cm.__enter__()
        ps01_cm = tc.tile_pool(name="ps01", bufs=1, space="PSUM")
        ps01 = ps01_cm.__enter__()

        ckv_sb = pA.tile([P, SKV], F32, tag="ckv")
        skv_sb = pA.tile([P, SKV], F32, tag="skv")
        x_bf = pA.tile([P, KT, SKV], FP8, tag="xbf")
        d1b = pA.tile([P, SKV], F32, tag="d1b")
        d1s = pA.tile([P, SKV], F32, tag="d1s")
        d1t = pA.tile([P, SKV // P], F32, tag="d1t")   # token-major d1

        s1a = ps01.tile([P, 512], F32, tag="s1a")
        s1b = ps01.tile([P, 512], F32, tag="s1b")
        # x chunk 0 gates the whole stats chain -- DMA it before everything else
        nc.sync.dma_start(x_bf[:, 0, :], d_xkv[:, 0, :])
        nc.sync.dma_start(ones_pp[:], d_ones[:])
        nc.sync.dma_start(consts[:], d_consts[:])
        nc.sync.dma_start(mask_sb[:], d_mask[:])
        for k in range(KT):
            if k > 0:
                nc.sync.dma_start(x_bf[:, k, :], d_xkv[:, k, :])
            sqv = tpE.tile([P, SKV], BF16, tag="sqkv")
            nc.vector.tensor_mul(out=sqv[:], in0=x_bf[:, k, :], in1=x_bf[:, k, :])
            nc.tensor.matmul(s1a[:], ones_pp[:], sqv[:, 0:512],
                             start=(k == 0), stop=(k == KT - 1))
            nc.tensor.matmul(s1b[:], ones_pp[:], sqv[:, 512:1024],
                             start=(k == 0), stop=(k == KT - 1))
        # tables DMA'd after the x stream so they don't delay the first matmuls
        nc.sync.dma_start(ckv_sb[:], d_ckv[:])
        nc.sync.dma_start(skv_sb[:], d_skv[:])
        # d1b = c_qk/sqrt(s/H + eps): the fp8 dequant constant is folded into
        # the sqrt scale/bias (divided by c^2), so no extra op is needed.
        # consts[:,0] = SA/(H*s_x^2*c^2), consts[:,1] = EPS/c^2.
        for half, ps in ((0, s1a), (1, s1b)):
            sl = slice(half * 512, (half + 1) * 512)
            nc.scalar.activation(d1s[:, sl], ps[:], AFT.Sqrt,
                                 bias=consts[:, 1:2], scale=consts[:, 0:1])
            scr = tpE.tile([P, 512], F32, tag="d1scr")
            nc.vector.reciprocal_approx_accurate(out=d1b[:, sl], in_=d1s[:, sl],
                                                 scratch=scr[:])
        ps01_cm.__exit__(None, None, None)
        # fold the per-token d1*c into the rope tables (saves one DVE mul per tile)
        nc.vector.tensor_mul(out=ckv_sb[:], in0=ckv_sb[:], in1=d1b[:])
        nc.vector.tensor_mul(out=skv_sb[:], in0=skv_sb[:], in1=d1b[:])
        eye_sb = pA.tile([P, P], F32, tag="eye")
        nc.sync.dma_start(eye_sb[:], d_eye[:])

        psmm1_cm = tc.tile_pool(name="psmm1", bufs=6, space="PSUM")
        psmm1 = psmm1_cm.__enter__()

        # ---- Phase 1: Q/K/V projections (+rope), fp8 DoubleRow ----
        qkv_cm = tc.tile_pool(name="qkv", bufs=1, side="right")
        qp_ = qkv_cm.__enter__()
        q_fm = qp_.tile([P, NH, T], BF16, tag="qfm")
        k_fm = qp_.tile([P, KVH, SKV], BF16, tag="kfm")
        v_tm = qp_.tile([P, SKV // P, DV], BF16, tag="vtm")

        def rope_out(ps, cos_t, sin_t, out_ap, n):
            # out = raw*cos' + swap_halves(raw)*sin'  (d1, dequant + sin sign
            # pre-folded into the tables). The cos term reads PSUM directly so
            # the psum tile is released after [copy, mul] and never waits for
            # the swap DMA round-trip.
            raw = tpE.tile([P, n], BF16, tag="rope_raw")
            nc.scalar.copy(raw[:], ps[:])
            rawc = tpE.tile([P, n], F32, tag="rope_rc")
            nc.vector.tensor_mul(out=rawc[:], in0=raw[:], in1=cos_t)
            sw = tpE.tile([P, n], BF16, tag="rope_sw")
            # issue the tiny half-swap DMAs from the ACT engine's DGE queue so
            # they don't convoy behind megabyte weight transfers on the sync
            # queue (that latency stalls DVE -> ACT -> attention psum handoff)
            nc.scalar.dma_start(sw[0:64, :], raw[64:128, :])
            nc.scalar.dma_start(sw[64:128, :], raw[0:64, :])
            nc.vector.tensor_mul(out=sw[:], in0=sw[:], in1=sin_t)
            nc.vector.tensor_add(out=out_ap, in0=rawc[:], in1=sw[:])

        # Q: 16 heads; q tokens are x_bf columns [Q0 : Q0+T)
        for mb in range(4):
            wt = wp8.tile([P, KT, 512], FP8, tag="wchunk")
            nc.sync.dma_start(wt[:], d_wq[mb])
            for ms in range(4):
                h = mb * 4 + ms
                ps = psmm1.tile([P, T], F32, tag="mm")
                for k in range(0, KT, 2):
                    nc.tensor.matmul(ps[:], wt[:, k:k + 2, ms * P:(ms + 1) * P],
                                     x_bf[:, k:k + 2, Q0:Q0 + T],
                                     start=(k == 0), stop=(k == KT - 2),
                                     perf_mode=DR)
                rope_out(ps, ckv_sb[:, Q0:Q0 + T], skv_sb[:, Q0:Q0 + T],
                         q_fm[:, h, :], T)

        # K: 4 kv heads x 2 halves of the kv sequence
        wtk = wp8.tile([P, KT, 512], FP8, tag="wchunk")
        nc.sync.dma_start(wtk[:], d_wk[:])
        for kvh in range(KVH):
            for half in range(2):
                ps = psmm1.tile([P, T], F32, tag="mm")
                for k in range(0, KT, 2):
                    nc.tensor.matmul(ps[:], wtk[:, k:k + 2, kvh * P:(kvh + 1) * P],
                                     x_bf[:, k:k + 2, half * 512:(half + 1) * 512],
                                     start=(k == 0), stop=(k == KT - 2),
                                     perf_mode=DR)
                rope_out(ps, ckv_sb[:, half * 512:(half + 1) * 512],
                         skv_sb[:, half * 512:(half + 1) * 512],
                         k_fm[:, kvh, half * 512:(half + 1) * 512], 512)

        # token-major d1 (for scaling V rows): PE-transpose each 128-col block
        # of the (row-broadcast) d1b and keep one column. Runs here (not in
        # phase 0) so the Q/K matmuls above aren't head-of-line blocked on the
        # d1 chain.
        for kvt in range(SKV // P):
            tps = psmm1.tile([P, P], F32, tag="tr", bufs=2)
            nc.tensor.transpose(tps[:], d1b[:, kvt * P:(kvt + 1) * P], eye_sb[:])
            nc.scalar.copy(d1t[:, kvt:kvt + 1], tps[:, 0:1])

        # V: token-major directly (lhsT = activations, rhs = weights)
        wtv = wp8.tile([P, KT, 512], FP8, tag="wchunk")
        nc.sync.dma_start(wtv[:], d_wv[:])
        for kvt in range(SKV // P):
            ps = psmm1.tile([P, DV], F32, tag="mm")
            for k in range(0, KT, 2):
                nc.tensor.matmul(ps[:], x_bf[:, k:k + 2, kvt * P:(kvt + 1) * P],
                                 wtv[:, k:k + 2, :],
                                 start=(k == 0), stop=(k == KT - 2),
                                 perf_mode=DR)
            # rows are tokens: apply per-token d1*c as a per-partition ACT scale
            nc.scalar.mul(v_tm[:, kvt, :], ps[:], d1t[:, kvt:kvt + 1])

        pA_cm.__exit__(None, None, None)
        psmm1_cm.__exit__(None, None, None)

        # ---- Phase 2: attention ----
        attn_cm = tc.tile_pool(name="attn", bufs=1)
        ap_ = attn_cm.__enter__()
        exp_cm = tc.tile_pool(name="exp", bufs=2)
        ep = exp_cm.__enter__()
        ps2_cm = tc.tile_pool(name="ps2", bufs=1, space="PSUM")
        ps2 = ps2_cm.__enter__()

        attn_cat = ap_.tile([P, NH, T], FP8, tag="attncat")

        # software pipeline: scores/exp of chain i+1 are emitted before the
        # PV/denominator matmuls of chain i, so the PE never head-of-line
        # blocks on the ACT exp latency
        def emit_sc(h, b):
            kvh = h // REP
            nkv = KVT_A if b == 0 else KVT_B
            moff = 0 if b == 0 else KVT_A
            qs = q_fm[:, h, b * BLK:(b + 1) * BLK]
            eb = ep.tile([P, KVT_B, BLK], BF16, tag="exp", bufs=4, name="eb")
            for g in range(nkv // 4):
                # 4 scores tiles into one 2-bank psum -> one batched exp
                sc4 = ps2.tile([P, 4, BLK], F32, tag="sc4", bufs=2, name="sc4")
                for j in range(4):
                    kvt = g * 4 + j
                    nc.tensor.matmul(sc4[:, j, :],
                                     k_fm[:, kvh, kvt * P:(kvt + 1) * P], qs)
                nc.scalar.activation(eb[:, g * 4:(g + 1) * 4, :], sc4[:],
                                     AFT.Exp, scale=SCALE)
                if b == 0 or g == 1:
                    # block B kv tiles 0-3 are causally clean on every core
                    nc.vector.tensor_mul(
                        out=eb[:, g * 4:(g + 1) * 4, :],
                        in0=eb[:, g * 4:(g + 1) * 4, :],
                        in1=mask_sb[:, moff + g * 4:moff + (g + 1) * 4, :])
            return (h, b, nkv, eb)

        def emit_pv(st):
            h, b, nkv, eb = st
            kvh = h // REP
            aps = ps2.tile([P, BLK], F32, tag="attnps", bufs=2, name="aps")
            dps = ps2.tile([P, BLK], F32, tag="denps", bufs=2, name="dps")
            for kvt in range(nkv):
                nc.tensor.matmul(aps[:], v_tm[:, kvt, kvh * P:(kvh + 1) * P],
                                 eb[:, kvt, :],
                                 start=(kvt == 0), stop=(kvt == nkv - 1))
                nc.tensor.matmul(dps[:], ones_pp[:], eb[:, kvt, :],
                                 start=(kvt == 0), stop=(kvt == nkv - 1))
            rec = tpE.tile([P, BLK], F32, tag="rec", bufs=3, name="rec")
            # ~18 correct bits -- plenty for a softmax denominator
            nc.vector.reciprocal_approx_fast(out=rec[:], in_=dps[:])
            # rec = SA/den, so attn_cat = SA*attn fits fp8 comfortably
            nc.vector.tensor_mul(out=attn_cat[:, h, b * BLK:(b + 1) * BLK],
                                 in0=aps[:], in1=rec[:])

        prev = None
        for h in range(NH):
            for b in range(2):
                st = emit_sc(h, b)
                if prev is not None:
                    emit_pv(prev)
                prev = st
        emit_pv(prev)

        exp_cm.__exit__(None, None, None)
        qkv_cm.__exit__(None, None, None)
        ps2_cm.__exit__(None, None, None)

        # ---- Phase 3: o_proj (fp8 DoubleRow) + residual + ln2 ----
        late_cm = tc.tile_pool(name="late", bufs=1, side="right")
        lp = late_cm.__enter__()
        ps3_cm = tc.tile_pool(name="ps3", bufs=1, space="PSUM")
        ps3 = ps3_cm.__enter__()

        h_res = lp.tile([P, KT, T], F32, tag="hres")
        mlp_in = lp.tile([P, KT, T], BF16, tag="mlpin")
        mlp_in8 = lp.tile([P, KT, T], FP8, tag="mlpin8")
        d2b8 = lp.tile([P, T], F32, tag="d2b8")

        s2 = ps3.tile([P, 512], F32, tag="s2")
        for mb in range(4):
            wt = wp8.tile([P, KT, 512], FP8, tag="wchunk")
            nc.sync.dma_start(wt[:], d_wo[mb])
            for ms in range(4):
                mt = mb * 4 + ms
                ps = ps3.tile([P, T], F32, tag="mm", bufs=4)
                for k in range(0, KT, 2):
                    nc.tensor.matmul(ps[:], wt[:, k:k + 2, ms * P:(ms + 1) * P],
                                     attn_cat[:, k:k + 2, :],
                                     start=(k == 0), stop=(k == KT - 2),
                                     perf_mode=DR)
                xres = tpE.tile([P, T], F32, tag="xres")
                nc.sync.dma_start(xres[:], d_xqres[:, mt, :])
                # h = ps * (1/(SA*s_wo)) + x   (fp8 dequant fused into the add)
                nc.vector.scalar_tensor_tensor(
                    out=h_res[:, mt, :], in0=ps[:], scalar=consts[:, 2:3],
                    in1=xres[:], op0=MUL, op1=ADD)
                sq2 = tpE.tile([P, T], BF16, tag="sqq")
                nc.vector.tensor_mul(out=sq2[:], in0=h_res[:, mt, :],
                                     in1=h_res[:, mt, :])
                nc.tensor.matmul(s2[:], ones_pp[:], sq2[:],
                                 start=(mt == 0), stop=(mt == KT - 1))
        # d2 = 1/sqrt(s2*SA/H + eps); sqrt reads psum directly, scale/bias fused
        d2s = tpE.tile([P, T], F32, tag="d2s")
        nc.scalar.activation(d2s[:], s2[:], AFT.Sqrt, bias=consts[:, 3:4],
                             scale=SA / H)
        d2scr = tpE.tile([P, T], F32, tag="d2scr")
        nc.vector.reciprocal_approx_accurate(out=d2b[:], in_=d2s[:],
                                             scratch=d2scr[:])
        for k in range(KT):
            nc.vector.tensor_mul(out=mlp_in[:, k, :], in0=h_res[:, k, :], in1=d2b[:])
        # fp8 copy of mlp_in (scaled by S_MI) for the fp8 gate_up chunks;
        # emitted after the bf16 tiles so the bf16 chunks start first
        nc.scalar.mul(d2b8[:], d2b[:], consts[:, 6:7])
        for k in range(KT):
            nc.vector.tensor_mul(out=mlp_in8[:, k, :], in0=h_res[:, k, :], in1=d2b8[:])

        attn_cm.__exit__(None, None, None)
        ps3_cm.__exit__(None, None, None)
        tpE_cm.__exit__(None, None, None)
        wp8_cm.__exit__(None, None, None)

        # ---- Phase 4: gate_up + SwiGLU ----
        mlp_cm = tc.tile_pool(name="mlp", bufs=1)
        mp = mlp_cm.__enter__()
        ps45_cm = tc.tile_pool(name="ps45", bufs=6, space="PSUM")
        ps45 = ps45_cm.__enter__()

        mid = mp.tile([P, IT, T], BF16, tag="mid")
        mid8 = mp.tile([P, 2 * N8, T], FP8, tag="mid8")
        # chunk mb columns: [gate[mb*256:(mb+1)*256], up[mb*256:(mb+1)*256]]
        for mb in range(32):
            f8 = mb >= NBF
            if f8:
                wt = mp.tile([P, KT, 512], FP8, tag="wchunk8", bufs=2)
                nc.sync.dma_start(wt[:], d_wgu8[mb - NBF])
            else:
                wt = wp.tile([P, KT, 512], BF16, tag="wchunk")
                nc.sync.dma_start(wt[:], d_wgu[mb])
            pss = []
            for ms in range(4):
                ps = ps45.tile([P, T], F32, tag="mm")
                if f8:
                    for k in range(0, KT, 2):
                        nc.tensor.matmul(ps[:], wt[:, k:k + 2, ms * P:(ms + 1) * P],
                                         mlp_in8[:, k:k + 2, :],
                                         start=(k == 0), stop=(k == KT - 2),
                                         perf_mode=DR)
                else:
                    for k in range(KT):
                        nc.tensor.matmul(ps[:], wt[:, k, ms * P:(ms + 1) * P],
                                         mlp_in[:, k, :],
                                         start=(k == 0), stop=(k == KT - 1))
                pss.append(ps)
            for j in range(2):
                # silu(g)*u = sigmoid(g)*g*u  (Silu table not in CoreSim)
                sg = tp.tile([P, T], F32, tag="silu")
                if f8:
                    # psum carries s_mi*s_wgu8; dequant c1 folded into each op
                    nc.scalar.activation(sg[:], pss[j][:], AFT.Sigmoid,
                                         scale=consts[:, 4:5])
                    t2 = tp.tile([P, T], F32, tag="silu2")
                    nc.vector.scalar_tensor_tensor(
                        out=t2[:], in0=pss[j][:], scalar=consts[:, 4:5],
                        in1=sg[:], op0=MUL, op1=MUL)
                    nc.vector.scalar_tensor_tensor(
                        out=mid8[:, 2 * (mb - NBF) + j, :], in0=pss[2 + j][:],
                        scalar=consts[:, 5:6], in1=t2[:], op0=MUL, op1=MUL)
                else:
                    nc.scalar.activation(sg[:], pss[j][:], AFT.Sigmoid)
                    t2 = tp.tile([P, T], F32, tag="silu2")
                    nc.vector.tensor_mul(out=t2[:], in0=sg[:], in1=pss[j][:])
                    nc.vector.tensor_mul(out=mid[:, 2 * mb + j, :], in0=t2[:],
                                         in1=pss[2 + j][:])

        # ---- Phase 5: down proj + residual ----
        for mt in range(KT):
            wt = wp.tile([P, ITBF, P], BF16, tag="wchunk")
            nc.sync.dma_start(wt[:], d_wd[mt])
            wt8 = mp.tile([P, 2 * N8, P], FP8, tag="wd8chunk", bufs=2)
            nc.sync.dma_start(wt8[:], d_wd8[mt])
            ps = ps45.tile([P, T], F32, tag="mm")
            for k in range(ITBF):
                nc.tensor.matmul(ps[:], wt[:, k, :], mid[:, k, :],
                                 start=(k == 0), stop=(k == ITBF - 1))
            ps8 = ps45.tile([P, T], F32, tag="mm")
            for k in range(0, 2 * N8, 2):
                nc.tensor.matmul(ps8[:], wt8[:, k:k + 2, :],
                                 mid8[:, k:k + 2, :],
                                 start=(k == 0), stop=(k == 2 * N8 - 2),
                                 perf_mode=DR)
            yt = tp.tile([P, T], F32, tag="yt")
            nc.vector.tensor_add(out=yt[:], in0=ps[:], in1=h_res[:, mt, :])
            yt2 = tp.tile([P, T], F32, tag="yt2")
            nc.vector.scalar_tensor_tensor(
                out=yt2[:], in0=ps8[:], scalar=consts[:, 7:8], in1=yt[:],
                op0=MUL, op1=ADD)
            nc.sync.dma_start(d_y[:, mt, :], yt2[:])

        mlp_cm.__exit__(None, None, None)
        ps45_cm.__exit__(None, None, None)
        late_cm.__exit__(None, None, None)
        tp_cm.__exit__(None, None, None)
        wp_cm.__exit__(None, None, None)
        glob_cm.__exit__(None, None, None)

    nc.compile()
    return nc


# ---------------- host-side preparation ----------------

def _perm(half):
    # q tokens sit at perm[Q0:Q0+T); block A's causal prefix fits in perm[0:512)
    if half == 0:
        return np.concatenate([np.arange(256, 512), np.arange(0, 256),
                               np.arange(768, 1024), np.arange(512, 768)])
    return np.arange(SKV)


def _pack_w(WT, mcol):
    # WT [K, M] -> [M//mcol, 128, K//128, mcol]; arr[mb,p,k,m] = WT[k*128+p, mb*mcol+m]
    K, M = WT.shape
    a = WT.reshape(K // P, P, M // mcol, mcol).transpose(2, 1, 0, 3)
    return np.ascontiguousarray(a)


def _q8(a, s):
    return np.clip(a * s, -240.0, 240.0).astype(F8)


def _prep_shared(inputs):
    w_ln1 = np.asarray(inputs["w_ln1"], np.float32)
    w_ln2 = np.asarray(inputs["w_ln2"], np.float32)
    w_q = np.asarray(inputs["w_q"], np.float32) * w_ln1[None, :]
    w_k = np.asarray(inputs["w_k"], np.float32) * w_ln1[None, :]
    w_v = np.asarray(inputs["w_v"], np.float32) * w_ln1[None, :]
    w_o = np.asarray(inputs["w_o"], np.float32)
    w_gu = np.asarray(inputs["w_gate_up"], np.float32) * w_ln2[None, :]
    w_d = np.asarray(inputs["w_down"], np.float32)

    x = np.asarray(inputs["hidden_states"], np.float32)
    s_x = 240.0 / max(float(np.abs(x).max()), 1e-30)
    wmax = max(float(np.abs(w_q).max()), float(np.abs(w_k).max()),
               float(np.abs(w_v).max()))
    s_w = 240.0 / max(wmax, 1e-30)
    s_wo = 240.0 / max(float(np.abs(w_o).max()), 1e-30)
    c_qk = 1.0 / (s_x * s_w)

    wq = _q8(_pack_w(w_q.T, 512), s_w)                  # [4,128,16,512]
    wk = _q8(_pack_w(w_k.T, 512)[0], s_w)               # [128,16,512]
    wv = _q8(_pack_w(w_v.T, 512)[0], s_w)
    wo = _q8(_pack_w(w_o.T, 512), s_wo)
    # gate/up interleave: chunk mb = [gate cols mb*256..], [up cols mb*256..]
    WT_gu = w_gu.T                                      # [H, 2I]
    cols = np.empty((32, 512), np.int64)
    for mb in range(32):
        cols[mb, :256] = np.arange(mb * 256, (mb + 1) * 256)
        cols[mb, 256:] = I + np.arange(mb * 256, (mb + 1) * 256)
    wgu_all = _pack_w(np.ascontiguousarray(WT_gu[:, cols.reshape(-1)]), 512)
    wgu = wgu_all[:NBF].astype(BF)
    wgu8 = _q8(wgu_all[NBF:], 240.0 / max(float(np.abs(w_gu).max()), 1e-30))
    wd_all = _pack_w(w_d.T, 128)                        # [16,128,64,128]
    wd = np.ascontiguousarray(wd_all[:, :, :ITBF, :]).astype(BF)
    wd8 = _q8(np.ascontiguousarray(wd_all[:, :, ITBF:, :]),
              240.0 / max(float(np.abs(w_d).max()), 1e-30))

    s_wgu8 = 240.0 / max(float(np.abs(w_gu).max()), 1e-30)
    s_wd8 = 240.0 / max(float(np.abs(w_d).max()), 1e-30)
    consts = np.empty((P, 8), np.float32)
    consts[:, 0] = SA / (H * s_x * s_x * c_qk * c_qk)
    consts[:, 1] = EPS / (c_qk * c_qk)
    consts[:, 2] = 1.0 / (SA * s_wo)
    consts[:, 3] = EPS
    consts[:, 4] = 1.0 / (S_MI * s_wgu8)
    consts[:, 5] = S_MID / (S_MI * s_wgu8)
    consts[:, 6] = S_MI
    consts[:, 7] = 1.0 / (S_MID * s_wd8)

    sin_t = np.asarray(inputs["sin_table"], np.float32)   # [S, 64]
    cos_t = np.asarray(inputs["cos_table"], np.float32)

    def rope_tables(pos):
        C = np.empty((P, len(pos)), np.float32)
        Sg = np.empty((P, len(pos)), np.float32)
        c = cos_t[pos, :].T                      # [64, n]
        s = sin_t[pos, :].T
        C[0:64] = c
        C[64:128] = c
        Sg[0:64] = -s
        Sg[64:128] = s
        return C, Sg

    per_half = {}
    for half in range(2):
        perm = _perm(half)
        C, Sg = rope_tables(perm)
        qpos = perm[Q0:Q0 + T]
        m = np.zeros((P, NMSK, BLK), np.float32)
        for b in range(2):
            qpb = qpos[b * BLK:(b + 1) * BLK]
            nkv = KVT_A if b == 0 else KVT_B
            moff = 0 if b == 0 else KVT_A
            for kvt in range(nkv):
                kvp = perm[kvt * P:(kvt + 1) * P]
                m[:, moff + kvt, :] = (kvp[:, None] <= qpb[None, :])
        per_half[half] = dict(perm=perm, ckv=C, skv=Sg, mask=m.astype(BF))

    ones_pp = np.full((P, P), 1.0 / SA, BF)
    eye_pp = np.eye(P, dtype=np.float32)
    return dict(wq=wq, wk=wk, wv=wv, wo=wo, wgu=wgu, wgu8=wgu8, wd=wd,
                wd8=wd8, s_x=s_x, per_half=per_half, ones_pp=ones_pp,
                eye_pp=eye_pp, consts=consts)


def _core_in_map(shared, x, core):
    b, half = core // 2, core % 2
    ph = shared["per_half"][half]
    xT = x[b].T[:, ph["perm"]]                           # [H, SKV] permuted
    x_pack = np.ascontiguousarray(xT.reshape(KT, P, SKV).transpose(1, 0, 2))
    return {
        "x_kv": _q8(x_pack, shared["s_x"]),
        "x_qres": np.ascontiguousarray(x_pack[:, :, Q0:Q0 + T], np.float32),
        "ckv": ph["ckv"], "skv": ph["skv"], "mask": ph["mask"],
        "ones_pp": shared["ones_pp"], "eye_pp": shared["eye_pp"],
        "consts": shared["consts"],
        "wq": shared["wq"], "wk": shared["wk"], "wv": shared["wv"],
        "wo": shared["wo"], "wgu": shared["wgu"], "wgu8": shared["wgu8"],
        "wd": shared["wd"], "wd8": shared["wd8"],
    }


_NC = None


def kernel(**inputs):
    global _NC, LAST_RESULT
    if _NC is None:
        _NC = build_nc()
    nc = _NC

    shared = _prep_shared(inputs)
    x = np.asarray(inputs["hidden_states"], np.float32)    # [B,S,H]
    in_maps = [_core_in_map(shared, x, c) for c in range(N_CORES)]

    trace = bool(int(os.environ.get("BASS_TRACE", "0") or "0"))
    res = None
    for attempt in range(3):
        try:
            res = run_bass_kernel_spmd(nc, in_maps, core_ids=list(range(N_CORES)),
                                       trace=trace)
            break
        except Exception:
            # the axon terminal occasionally wedges transiently (LoadExecutable
            # failures); it recovers after a short idle
            if attempt == 2:
                raise
            import time
            time.sleep(90)
    LAST_RESULT = res

    out = np.empty((B, S, H), np.float32)
    for c in range(N_CORES):
        b, half = c // 2, c % 2
        qpos = _perm(half)[Q0:Q0 + T]
        y = res.results[c]["y"]                            # [128,16,512]
        out[b, qpos, :] = y.transpose(1, 0, 2).reshape(H, T).T
    return out


# revision 23
# speedup vs baseline: 1.0764x; 1.0073x over previous
"""Trainium2 Bass kernel for a dense decoder layer (RMSNorm -> GQA attn -> RMSNorm -> SwiGLU MLP).

Sharding: token-parallel across 8 cores (no collectives). Each core owns 512
query tokens of one batch (two causally balanced 256-token blocks) and computes
K/V for its batch's full 1024 tokens. Host permutes the batch's token columns
per core so q tokens always sit at columns [256:768) and block A's causal
prefix fits in kv columns [0:512) -- the compiled program is identical on all
cores. Activations are feature-major [feature_part, token_free] so matmuls
chain with no transposes; V is produced token-major by swapping matmul operand
roles.

Precision: Q/K/V/O projections run in fp8e4 (DoubleRow perf mode, 2 k-tiles
per matmul) with fp32 PSUM accumulation; x and those four weights are
quantized host-side with per-tensor absmax scales. The dequant constant
1/(s_x*s_w) is folded into the rmsnorm-1 rsqrt (imm/eps divided by c^2), so
d1b carries it into the rope tables and V's per-token scale for free. The
attention output is written as fp8 scaled by s_a=32 (exact in bf16), with
1/s_a folded into the all-ones stats/denominator matmul weight and the
residual add carrying 1/(s_a*s_wo). Scores/PV and the whole MLP stay bf16
(fp8 there busts the 2e-2 error budget). Runtime-dependent dequant scalars
ride in a tiny "consts" input tensor since immediates are compile-time.

Softmax skips max-subtraction (|scores| < ~6) and causality is a host-built
0/1 mask multiplied after exp. Partition-axis reductions use an all-[1/32]
[128,128] stationary matmul, which also broadcasts results across partitions.
"""

import os

import numpy as np
import ml_dtypes

import concourse.bass as bass  # noqa: F401
import concourse.mybir as mybir
import concourse.tile as tile
from concourse import bacc
from concourse.bass_utils import run_bass_kernel_spmd

# ---- problem shapes (hardcoded) ----
B, S, H = 4, 1024, 2048
NH, KVH, HD = 16, 4, 128
I = 8192
EPS = 1e-6

P = 128
KT = H // P            # 16 k-tiles over H
T = 512                # q tokens per core
SKV = 1024             # kv tokens per core (its batch's full sequence)
DV = KVH * HD          # 512
REP = NH // KVH
N_CORES = 8
BLK = 256              # q block size
KVT_A, KVT_B = 4, 8    # kv tiles processed for block A / block B
NMSK = KVT_A + KVT_B   # 12
IT = I // P            # 64 k-tiles over I
Q0 = 256               # q tokens live at columns [Q0 : Q0+T)
SCALE = 1.0 / float(np.sqrt(HD))
SA = 32.0              # attn fp8 scale; 1/32 is exact in bf16
N8 = 5                 # gate_up chunks (of 32) run in fp8; tail of the I dim
NBF = 32 - N8          # bf16 gate_up chunks
ITBF = IT - 2 * N8     # bf16 down-proj k-tiles
S_MI = 15.0            # mlp_in fp8 scale (|mlp_in| < 16 by rmsnorm bound)
S_MID = 5.0            # mid fp8 scale (|mid| < 48)

F32 = mybir.dt.float32
BF16 = mybir.dt.bfloat16
FP8 = mybir.dt.float8e4
MUL = mybir.AluOpType.mult
ADD = mybir.AluOpType.add
AFT = mybir.ActivationFunctionType
DR = mybir.MatmulPerfMode.DoubleRow

BF = ml_dtypes.bfloat16
F8 = ml_dtypes.float8_e4m3fn

LAST_RESULT = None  # BassKernelResults of the most recent run (for test harness)


def _install_ntff_hook():
    """The agent image's `antenv` lacks `axon_hooks`, so the boot shim's NTFF
    hook registration degrades silently and bass_utils crashes on import of
    antenv.axon_hooks when trace=True. Recreate the module and register the
    ctypes-based hook from trn_agent_boot."""
    import sys
    import types
    try:
        import antenv.axon_hooks  # noqa: F401
        return
    except ImportError:
        pass
    try:
        import antenv
    except ImportError:
        return
    mod = types.ModuleType("antenv.axon_hooks")
    _hook = [None]
    mod.set_axon_ntff_profile_hook = lambda h: _hook.__setitem__(0, h)
    mod.get_axon_ntff_profile_hook = lambda: _hook[0]
    sys.modules["antenv.axon_hooks"] = mod
    antenv.axon_hooks = mod
    try:
        from trn_agent_boot import trn_boot
        import ctypes
        so_path = "/opt/axon/libaxon_pjrt.so"
        lib = ctypes.CDLL(so_path)
        if hasattr(lib, "axon_start_nrt_profile"):
            mod.set_axon_ntff_profile_hook(
                trn_boot._ntff_profile_via_ctypes(so_path))
    except Exception:
        pass


_install_ntff_hook()


def build_nc():
    nc = bacc.Bacc(
        "TRN2",
        target_bir_lowering=False,
        debug=False,
        enable_asserts=False,
        num_devices=N_CORES,
    )

    # ---- DRAM I/O ----
    d_xkv = nc.dram_tensor("x_kv", [P, KT, SKV], FP8, kind="ExternalInput")
    d_xqres = nc.dram_tensor("x_qres", [P, KT, T], F32, kind="ExternalInput")
    d_ckv = nc.dram_tensor("ckv", [P, SKV], F32, kind="ExternalInput")
    d_skv = nc.dram_tensor("skv", [P, SKV], F32, kind="ExternalInput")
    d_consts = nc.dram_tensor("consts", [P, 8], F32, kind="ExternalInput")
    d_mask = nc.dram_tensor("mask", [P, NMSK, BLK], BF16, kind="ExternalInput")
    d_ones = nc.dram_tensor("ones_pp", [P, P], BF16, kind="ExternalInput")
    d_eye = nc.dram_tensor("eye_pp", [P, P], F32, kind="ExternalInput")
    d_wq = nc.dram_tensor("wq", [4, P, KT, 512], FP8, kind="ExternalInput")
    d_wk = nc.dram_tensor("wk", [P, KT, 512], FP8, kind="ExternalInput")
    d_wv = nc.dram_tensor("wv", [P, KT, 512], FP8, kind="ExternalInput")
    d_wo = nc.dram_tensor("wo", [4, P, KT, 512], FP8, kind="ExternalInput")
    d_wgu = nc.dram_tensor("wgu", [NBF, P, KT, 512], BF16, kind="ExternalInput")
    d_wgu8 = nc.dram_tensor("wgu8", [N8, P, KT, 512], FP8, kind="ExternalInput")
    d_wd = nc.dram_tensor("wd", [16, P, ITBF, P], BF16, kind="ExternalInput")
    d_wd8 = nc.dram_tensor("wd8", [16, P, 2 * N8, P], FP8, kind="ExternalInput")
    d_y = nc.dram_tensor("y", [P, KT, T], F32, kind="ExternalOutput")

    with tile.TileContext(nc) as tc:
        # long-lived pools
        glob_cm = tc.tile_pool(name="glob", bufs=1)
        gp = glob_cm.__enter__()
        wp_cm = tc.tile_pool(name="wstream", bufs=2)
        wp = wp_cm.__enter__()
        tp_cm = tc.tile_pool(name="tmp", bufs=2)
        tp = tp_cm.__enter__()
        wp8_cm = tc.tile_pool(name="wstream8", bufs=2)
        wp8 = wp8_cm.__enter__()
        # scratch that dies with phase 3 (freed before the MLP pool opens)
        tpE_cm = tc.tile_pool(name="tmpE", bufs=2)
        tpE = tpE_cm.__enter__()

        ones_pp = gp.tile([P, P], BF16, tag="ones")
        consts = gp.tile([P, 8], F32, tag="consts")
        d2b = gp.tile([P, T], F32, tag="d2b")
        mask_sb = gp.tile([P, NMSK, BLK], BF16, tag="mask")

        # ---- Phase 0: RMSNorm1 stats (projections run on raw fp8 x; the
        # per-token scale d1 is applied post-matmul, so the PE never waits
        # for the stats chain) ----
        pA_cm = tc.tile_pool(name="ph01", bufs=1)
        pA = pA_cm.__enter__()
        ps01_cm = tc.tile_pool(name="ps01", bufs=1, space="PSUM")
        ps01 = ps01_cm.__enter__()

        ckv_sb = pA.tile([P, SKV], F32, tag="ckv")
        skv_sb = pA.tile([P, SKV], F32, tag="skv")
        x_bf = pA.tile([P, KT, SKV], FP8, tag="xbf")
        d1b = pA.tile([P, SKV], F32, tag="d1b")
        d1s = pA.tile([P, SKV], F32, tag="d1s")
        d1t = pA.tile([P, SKV // P], F32, tag="d1t")   # token-major d1

        s1a = ps01.tile([P, 512], F32, tag="s1a")
        s1b = ps01.tile([P, 512], F32, tag="s1b")
        # x chunk 0 gates the whole stats chain -- DMA it before everything else
        nc.sync.dma_start(x_bf[:, 0, :], d_xkv[:, 0, :])
        nc.sync.dma_start(ones_pp[:], d_ones[:])
        nc.sync.dma_start(consts[:], d_consts[:])
        # static tensors ride the ACT-engine DGE queue, off the x/weight path
        nc.scalar.dma_start(mask_sb[:], d_mask[:])
        for k in range(KT):
            if k > 0:
                nc.sync.dma_start(x_bf[:, k, :], d_xkv[:, k, :])
            sqv = tpE.tile([P, SKV], BF16, tag="sqkv")
            nc.vector.tensor_mul(out=sqv[:], in0=x_bf[:, k, :], in1=x_bf[:, k, :])
            nc.tensor.matmul(s1a[:], ones_pp[:], sqv[:, 0:512],
                             start=(k == 0), stop=(k == KT - 1))
            nc.tensor.matmul(s1b[:], ones_pp[:], sqv[:, 512:1024],
                             start=(k == 0), stop=(k == KT - 1))
        # tables ride the ACT queue so they don't delay the weight stream
        nc.scalar.dma_start(ckv_sb[:], d_ckv[:])
        nc.scalar.dma_start(skv_sb[:], d_skv[:])
        # d1b = c_qk/sqrt(s/H + eps): the fp8 dequant constant is folded into
        # the sqrt scale/bias (divided by c^2), so no extra op is needed.
        # consts[:,0] = SA/(H*s_x^2*c^2), consts[:,1] = EPS/c^2.
        for half, ps in ((0, s1a), (1, s1b)):
            sl = slice(half * 512, (half + 1) * 512)
            nc.scalar.activation(d1s[:, sl], ps[:], AFT.Sqrt,
                                 bias=consts[:, 1:2], scale=consts[:, 0:1])
            scr = tpE.tile([P, 512], F32, tag="d1scr")
            nc.vector.reciprocal_approx_accurate(out=d1b[:, sl], in_=d1s[:, sl],
                                                 scratch=scr[:])
        ps01_cm.__exit__(None, None, None)
        # fold the per-token d1*c into the rope tables (saves one DVE mul per tile)
        nc.vector.tensor_mul(out=ckv_sb[:], in0=ckv_sb[:], in1=d1b[:])
        nc.vector.tensor_mul(out=skv_sb[:], in0=skv_sb[:], in1=d1b[:])
        eye_sb = pA.tile([P, P], F32, tag="eye")
        nc.sync.dma_start(eye_sb[:], d_eye[:])

        psmm1_cm = tc.tile_pool(name="psmm1", bufs=6, space="PSUM")
        psmm1 = psmm1_cm.__enter__()

        # ---- Phase 1: Q/K/V projections (+rope), fp8 DoubleRow ----
        qkv_cm = tc.tile_pool(name="qkv", bufs=1, side="right")
        qp_ = qkv_cm.__enter__()
        q_fm = qp_.tile([P, NH, T], BF16, tag="qfm")
        k_fm = qp_.tile([P, KVH, SKV], BF16, tag="kfm")
        v_tm = qp_.tile([P, SKV // P, DV], BF16, tag="vtm")

        def rope_out(ps, cos_t, sin_t, out_ap, n):
            # out = raw*cos' + swap_halves(raw)*sin'  (d1, dequant + sin sign
            # pre-folded into the tables). The cos term reads PSUM directly so
            # the psum tile is released after [copy, mul] and never waits for
            # the swap DMA round-trip.
            raw = tpE.tile([P, n], BF16, tag="rope_raw")
            nc.scalar.copy(raw[:], ps[:])
            rawc = tpE.tile([P, n], F32, tag="rope_rc")
            nc.vector.tensor_mul(out=rawc[:], in0=raw[:], in1=cos_t)
            sw = tpE.tile([P, n], BF16, tag="rope_sw")
            # issue the tiny half-swap DMAs from the ACT engine's DGE queue so
            # they don't convoy behind megabyte weight transfers on the sync
            # queue (that latency stalls DVE -> ACT -> attention psum handoff)
            nc.scalar.dma_start(sw[0:64, :], raw[64:128, :])
            nc.scalar.dma_start(sw[64:128, :], raw[0:64, :])
            nc.vector.tensor_mul(out=sw[:], in0=sw[:], in1=sin_t)
            nc.vector.tensor_add(out=out_ap, in0=rawc[:], in1=sw[:])

        # Q: 16 heads; q tokens are x_bf columns [Q0 : Q0+T)
        for mb in range(4):
            wt = wp8.tile([P, KT, 512], FP8, tag="wchunk")
            nc.sync.dma_start(wt[:], d_wq[mb])
            for ms in range(4):
                h = mb * 4 + ms
                ps = psmm1.tile([P, T], F32, tag="mm")
                for k in range(0, KT, 2):
                    nc.tensor.matmul(ps[:], wt[:, k:k + 2, ms * P:(ms + 1) * P],
                                     x_bf[:, k:k + 2, Q0:Q0 + T],
                                     start=(k == 0), stop=(k == KT - 2),
                                     perf_mode=DR)
                rope_out(ps, ckv_sb[:, Q0:Q0 + T], skv_sb[:, Q0:Q0 + T],
                         q_fm[:, h, :], T)

        # K: 4 kv heads x 2 halves of the kv sequence
        wtk = wp8.tile([P, KT, 512], FP8, tag="wchunk")
        nc.sync.dma_start(wtk[:], d_wk[:])
        for kvh in range(KVH):
            for half in range(2):
                ps = psmm1.tile([P, T], F32, tag="mm")
                for k in range(0, KT, 2):
                    nc.tensor.matmul(ps[:], wtk[:, k:k + 2, kvh * P:(kvh + 1) * P],
                                     x_bf[:, k:k + 2, half * 512:(half + 1) * 512],
                                     start=(k == 0), stop=(k == KT - 2),
                                     perf_mode=DR)
                rope_out(ps, ckv_sb[:, half * 512:(half + 1) * 512],
                         skv_sb[:, half * 512:(half + 1) * 512],
                         k_fm[:, kvh, half * 512:(half + 1) * 512], 512)

        # token-major d1 (for scaling V rows): PE-transpose each 128-col block
        # of the (row-broadcast) d1b and keep one column. Runs here (not in
        # phase 0) so the Q/K matmuls above aren't head-of-line blocked on the
        # d1 chain.
        for kvt in range(SKV // P):
            tps = psmm1.tile([P, P], F32, tag="tr", bufs=2)
            nc.tensor.transpose(tps[:], d1b[:, kvt * P:(kvt + 1) * P], eye_sb[:])
            nc.scalar.copy(d1t[:, kvt:kvt + 1], tps[:, 0:1])

        # V: token-major directly (lhsT = activations, rhs = weights)
        wtv = wp8.tile([P, KT, 512], FP8, tag="wchunk")
        nc.sync.dma_start(wtv[:], d_wv[:])
        for kvt in range(SKV // P):
            ps = psmm1.tile([P, DV], F32, tag="mm")
            for k in range(0, KT, 2):
                nc.tensor.matmul(ps[:], x_bf[:, k:k + 2, kvt * P:(kvt + 1) * P],
                                 wtv[:, k:k + 2, :],
                                 start=(k == 0), stop=(k == KT - 2),
                                 perf_mode=DR)
            # rows are tokens: apply per-token d1*c as a per-partition ACT scale
            nc.scalar.mul(v_tm[:, kvt, :], ps[:], d1t[:, kvt:kvt + 1])

        pA_cm.__exit__(None, None, None)
        psmm1_cm.__exit__(None, None, None)

        # ---- Phase 2: attention ----
        attn_cm = tc.tile_pool(name="attn", bufs=1)
        ap_ = attn_cm.__enter__()
        exp_cm = tc.tile_pool(name="exp", bufs=2)
        ep = exp_cm.__enter__()
        ps2_cm = tc.tile_pool(name="ps2", bufs=1, space="PSUM")
        ps2 = ps2_cm.__enter__()

        attn_cat = ap_.tile([P, NH, T], FP8, tag="attncat")

        # software pipeline: scores/exp of chain i+1 are emitted before the
        # PV/denominator matmuls of chain i, so the PE never head-of-line
        # blocks on the ACT exp latency
        def emit_sc(h, b):
            kvh = h // REP
            nkv = KVT_A if b == 0 else KVT_B
            moff = 0 if b == 0 else KVT_A
            qs = q_fm[:, h, b * BLK:(b + 1) * BLK]
            eb = ep.tile([P, KVT_B, BLK], BF16, tag="exp", bufs=4, name="eb")
            for g in range(nkv // 4):
                # 4 scores tiles into one 2-bank psum -> one batched exp
                sc4 = ps2.tile([P, 4, BLK], F32, tag="sc4", bufs=2, name="sc4")
                for j in range(4):
                    kvt = g * 4 + j
                    nc.tensor.matmul(sc4[:, j, :],
                                     k_fm[:, kvh, kvt * P:(kvt + 1) * P], qs)
                nc.scalar.activation(eb[:, g * 4:(g + 1) * 4, :], sc4[:],
                                     AFT.Exp, scale=SCALE)
                if b == 0 or g == 1:
                    # block B kv tiles 0-3 are causally clean on every core
                    nc.vector.tensor_mul(
                        out=eb[:, g * 4:(g + 1) * 4, :],
                        in0=eb[:, g * 4:(g + 1) * 4, :],
                        in1=mask_sb[:, moff + g * 4:moff + (g + 1) * 4, :])
            return (h, b, nkv, eb)

        def emit_pv(st):
            h, b, nkv, eb = st
            kvh = h // REP
            aps = ps2.tile([P, BLK], F32, tag="attnps", bufs=2, name="aps")
            dps = ps2.tile([P, BLK], F32, tag="denps", bufs=2, name="dps")
            for kvt in range(nkv):
                nc.tensor.matmul(aps[:], v_tm[:, kvt, kvh * P:(kvh + 1) * P],
                                 eb[:, kvt, :],
                                 start=(kvt == 0), stop=(kvt == nkv - 1))
                nc.tensor.matmul(dps[:], ones_pp[:], eb[:, kvt, :],
                                 start=(kvt == 0), stop=(kvt == nkv - 1))
            rec = tpE.tile([P, BLK], F32, tag="rec", bufs=3, name="rec")
            # ~18 correct bits -- plenty for a softmax denominator
            nc.vector.reciprocal_approx_fast(out=rec[:], in_=dps[:])
            # rec = SA/den, so attn_cat = SA*attn fits fp8 comfortably
            nc.vector.tensor_mul(out=attn_cat[:, h, b * BLK:(b + 1) * BLK],
                                 in0=aps[:], in1=rec[:])

        prev = None
        for h in range(NH):
            for b in range(2):
                st = emit_sc(h, b)
                if prev is not None:
                    emit_pv(prev)
                prev = st
        emit_pv(prev)

        exp_cm.__exit__(None, None, None)
        qkv_cm.__exit__(None, None, None)
        ps2_cm.__exit__(None, None, None)

        # ---- Phase 3: o_proj (fp8 DoubleRow) + residual + ln2 ----
        late_cm = tc.tile_pool(name="late", bufs=1, side="right")
        lp = late_cm.__enter__()
        ps3_cm = tc.tile_pool(name="ps3", bufs=1, space="PSUM")
        ps3 = ps3_cm.__enter__()

        h_res = lp.tile([P, KT, T], F32, tag="hres")
        mlp_in = lp.tile([P, KT, T], BF16, tag="mlpin")
        mlp_in8 = lp.tile([P, KT, T], FP8, tag="mlpin8")
        d2b8 = lp.tile([P, T], F32, tag="d2b8")

        s2 = ps3.tile([P, 512], F32, tag="s2")
        for mb in range(4):
            wt = wp8.tile([P, KT, 512], FP8, tag="wchunk")
            nc.sync.dma_start(wt[:], d_wo[mb])
            for ms in range(4):
                mt = mb * 4 + ms
                ps = ps3.tile([P, T], F32, tag="mm", bufs=4)
                for k in range(0, KT, 2):
                    nc.tensor.matmul(ps[:], wt[:, k:k + 2, ms * P:(ms + 1) * P],
                                     attn_cat[:, k:k + 2, :],
                                     start=(k == 0), stop=(k == KT - 2),
                                     perf_mode=DR)
                xres = tpE.tile([P, T], F32, tag="xres")
                nc.sync.dma_start(xres[:], d_xqres[:, mt, :])
                # h = ps * (1/(SA*s_wo)) + x   (fp8 dequant fused into the add)
                nc.vector.scalar_tensor_tensor(
                    out=h_res[:, mt, :], in0=ps[:], scalar=consts[:, 2:3],
                    in1=xres[:], op0=MUL, op1=ADD)
                sq2 = tpE.tile([P, T], BF16, tag="sqq")
                nc.vector.tensor_mul(out=sq2[:], in0=h_res[:, mt, :],
                                     in1=h_res[:, mt, :])
                nc.tensor.matmul(s2[:], ones_pp[:], sq2[:],
                                 start=(mt == 0), stop=(mt == KT - 1))
        # d2 = 1/sqrt(s2*SA/H + eps); sqrt reads psum directly, scale/bias fused
        d2s = tpE.tile([P, T], F32, tag="d2s")
        nc.scalar.activation(d2s[:], s2[:], AFT.Sqrt, bias=consts[:, 3:4],
                             scale=SA / H)
        d2scr = tpE.tile([P, T], F32, tag="d2scr")
        nc.vector.reciprocal_approx_accurate(out=d2b[:], in_=d2s[:],
                                             scratch=d2scr[:])
        for k in range(KT):
            nc.vector.tensor_mul(out=mlp_in[:, k, :], in0=h_res[:, k, :], in1=d2b[:])
        # fp8 copy of mlp_in (scaled by S_MI) for the fp8 gate_up chunks;
        # emitted after the bf16 tiles so the bf16 chunks start first
        nc.scalar.mul(d2b8[:], d2b[:], consts[:, 6:7])
        for k in range(KT):
            nc.vector.tensor_mul(out=mlp_in8[:, k, :], in0=h_res[:, k, :], in1=d2b8[:])

        attn_cm.__exit__(None, None, None)
        ps3_cm.__exit__(None, None, None)
        tpE_cm.__exit__(None, None, None)
        wp8_cm.__exit__(None, None, None)

        # ---- Phase 4: gate_up + SwiGLU ----
        mlp_cm = tc.tile_pool(name="mlp", bufs=1)
        mp = mlp_cm.__enter__()
        ps45_cm = tc.tile_pool(name="ps45", bufs=6, space="PSUM")
        ps45 = ps45_cm.__enter__()

        mid = mp.tile([P, IT, T], BF16, tag="mid")
        mid8 = mp.tile([P, 2 * N8, T], FP8, tag="mid8")
        # chunk mb columns: [gate[mb*256:(mb+1)*256], up[mb*256:(mb+1)*256]]
        for mb in range(32):
            f8 = mb >= NBF
            if f8:
                wt = mp.tile([P, KT, 512], FP8, tag="wchunk8", bufs=2)
                nc.sync.dma_start(wt[:], d_wgu8[mb - NBF])
            else:
                wt = wp.tile([P, KT, 512], BF16, tag="wchunk")
                nc.sync.dma_start(wt[:], d_wgu[mb])
            pss = []
            for ms in range(4):
                ps = ps45.tile([P, T], F32, tag="mm")
                if f8:
                    for k in range(0, KT, 2):
                        nc.tensor.matmul(ps[:], wt[:, k:k + 2, ms * P:(ms + 1) * P],
                                         mlp_in8[:, k:k + 2, :],
                                         start=(k == 0), stop=(k == KT - 2),
                                         perf_mode=DR)
                else:
                    for k in range(KT):
                        nc.tensor.matmul(ps[:], wt[:, k, ms * P:(ms + 1) * P],
                                         mlp_in[:, k, :],
                                         start=(k == 0), stop=(k == KT - 1))
                pss.append(ps)
            for j in range(2):
                # silu(g)*u = sigmoid(g)*g*u  (Silu table not in CoreSim)
                sg = tp.tile([P, T], F32, tag="silu")
                if f8:
                    # psum carries s_mi*s_wgu8; dequant c1 folded into each op
                    nc.scalar.activation(sg[:], pss[j][:], AFT.Sigmoid,
                                         scale=consts[:, 4:5])
                    t2 = tp.tile([P, T], F32, tag="silu2")
                    nc.vector.scalar_tensor_tensor(
                        out=t2[:], in0=pss[j][:], scalar=consts[:, 4:5],
                        in1=sg[:], op0=MUL, op1=MUL)
                    nc.vector.scalar_tensor_tensor(
                        out=mid8[:, 2 * (mb - NBF) + j, :], in0=pss[2 + j][:],
                        scalar=consts[:, 5:6], in1=t2[:], op0=MUL, op1=MUL)
                else:
                    nc.scalar.activation(sg[:], pss[j][:], AFT.Sigmoid)
                    t2 = tp.tile([P, T], F32, tag="silu2")
                    nc.vector.tensor_mul(out=t2[:], in0=sg[:], in1=pss[j][:])
                    nc.vector.tensor_mul(out=mid[:, 2 * mb + j, :], in0=t2[:],
                                         in1=pss[2 + j][:])

        # ---- Phase 5: down proj + residual ----
        for mt in range(KT):
            wt = wp.tile([P, ITBF, P], BF16, tag="wchunk")
            nc.sync.dma_start(wt[:], d_wd[mt])
            wt8 = mp.tile([P, 2 * N8, P], FP8, tag="wd8chunk", bufs=2)
            nc.sync.dma_start(wt8[:], d_wd8[mt])
            ps = ps45.tile([P, T], F32, tag="mm")
            for k in range(ITBF):
                nc.tensor.matmul(ps[:], wt[:, k, :], mid[:, k, :],
                                 start=(k == 0), stop=(k == ITBF - 1))
            ps8 = ps45.tile([P, T], F32, tag="mm")
            for k in range(0, 2 * N8, 2):
                nc.tensor.matmul(ps8[:], wt8[:, k:k + 2, :],
                                 mid8[:, k:k + 2, :],
                                 start=(k == 0), stop=(k == 2 * N8 - 2),
                                 perf_mode=DR)
            yt = tp.tile([P, T], F32, tag="yt")
            nc.vector.tensor_add(out=yt[:], in0=ps[:], in1=h_res[:, mt, :])
            yt2 = tp.tile([P, T], F32, tag="yt2")
            nc.vector.scalar_tensor_tensor(
                out=yt2[:], in0=ps8[:], scalar=consts[:, 7:8], in1=yt[:],
                op0=MUL, op1=ADD)
            nc.sync.dma_start(d_y[:, mt, :], yt2[:])

        mlp_cm.__exit__(None, None, None)
        ps45_cm.__exit__(None, None, None)
        late_cm.__exit__(None, None, None)
        tp_cm.__exit__(None, None, None)
        wp_cm.__exit__(None, None, None)
        glob_cm.__exit__(None, None, None)

    nc.compile()
    return nc


# ---------------- host-side preparation ----------------

def _perm(half):
    # q tokens sit at perm[Q0:Q0+T); block A's causal prefix fits in perm[0:512)
    if half == 0:
        return np.concatenate([np.arange(256, 512), np.arange(0, 256),
                               np.arange(768, 1024), np.arange(512, 768)])
    return np.arange(SKV)


def _pack_w(WT, mcol):
    # WT [K, M] -> [M//mcol, 128, K//128, mcol]; arr[mb,p,k,m] = WT[k*128+p, mb*mcol+m]
    K, M = WT.shape
    a = WT.reshape(K // P, P, M // mcol, mcol).transpose(2, 1, 0, 3)
    return np.ascontiguousarray(a)


def _q8(a, s):
    return np.clip(a * s, -240.0, 240.0).astype(F8)


def _prep_shared(inputs):
    w_ln1 = np.asarray(inputs["w_ln1"], np.float32)
    w_ln2 = np.asarray(inputs["w_ln2"], np.float32)
    w_q = np.asarray(inputs["w_q"], np.float32) * w_ln1[None, :]
    w_k = np.asarray(inputs["w_k"], np.float32) * w_ln1[None, :]
    w_v = np.asarray(inputs["w_v"], np.float32) * w_ln1[None, :]
    w_o = np.asarray(inputs["w_o"], np.float32)
    w_gu = np.asarray(inputs["w_gate_up"], np.float32) * w_ln2[None, :]
    w_d = np.asarray(inputs["w_down"], np.float32)

    x = np.asarray(inputs["hidden_states"], np.float32)
    s_x = 240.0 / max(float(np.abs(x).max()), 1e-30)
    wmax = max(float(np.abs(w_q).max()), float(np.abs(w_k).max()),
               float(np.abs(w_v).max()))
    s_w = 240.0 / max(wmax, 1e-30)
    s_wo = 240.0 / max(float(np.abs(w_o).max()), 1e-30)
    c_qk = 1.0 / (s_x * s_w)

    wq = _q8(_pack_w(w_q.T, 512), s_w)                  # [4,128,16,512]
    wk = _q8(_pack_w(w_k.T, 512)[0], s_w)               # [128,16,512]
    wv = _q8(_pack_w(w_v.T, 512)[0], s_w)
    wo = _q8(_pack_w(w_o.T, 512), s_wo)
    # gate/up interleave: chunk mb = [gate cols mb*256..], [up cols mb*256..]
    WT_gu = w_gu.T                                      # [H, 2I]
    cols = np.empty((32, 512), np.int64)
    for mb in range(32):
        cols[mb, :256] = np.arange(mb * 256, (mb + 1) * 256)
        cols[mb, 256:] = I + np.arange(mb * 256, (mb + 1) * 256)
    wgu_all = _pack_w(np.ascontiguousarray(WT_gu[:, cols.reshape(-1)]), 512)
    wgu = wgu_all[:NBF].astype(BF)
    wgu8 = _q8(wgu_all[NBF:], 240.0 / max(float(np.abs(w_gu).max()), 1e-30))
    wd_all = _pack_w(w_d.T, 128)                        # [16,128,64,128]
    wd = np.ascontiguousarray(wd_all[:, :, :ITBF, :]).astype(BF)
    wd8 = _q8(np.ascontiguousarray(wd_all[:, :, ITBF:, :]),
              240.0 / max(float(np.abs(w_d).max()), 1e-30))

    s_wgu8 = 240.0 / max(float(np.abs(w_gu).max()), 1e-30)
    s_wd8 = 240.0 / max(float(np.abs(w_d).max()), 1e-30)
    consts = np.empty((P, 8), np.float32)
    consts[:, 0] = SA / (H * s_x * s_x * c_qk * c_qk)
    consts[:, 1] = EPS / (c_qk * c_qk)
    consts[:, 2] = 1.0 / (SA * s_wo)
    consts[:, 3] = EPS
    consts[:, 4] = 1.0 / (S_MI * s_wgu8)
    consts[:, 5] = S_MID / (S_MI * s_wgu8)
    consts[:, 6] = S_MI
    consts[:, 7] = 1.0 / (S_MID * s_wd8)

    sin_t = np.asarray(inputs["sin_table"], np.float32)   # [S, 64]
    cos_t = np.asarray(inputs["cos_table"], np.float32)

    def rope_tables(pos):
        C = np.empty((P, len(pos)), np.float32)
        Sg = np.empty((P, len(pos)), np.float32)
        c = cos_t[pos, :].T                      # [64, n]
        s = sin_t[pos, :].T
        C[0:64] = c
        C[64:128] = c
        Sg[0:64] = -s
        Sg[64:128] = s
        return C, Sg

    per_half = {}
    for half in range(2):
        perm = _perm(half)
        C, Sg = rope_tables(perm)
        qpos = perm[Q0:Q0 + T]
        m = np.zeros((P, NMSK, BLK), np.float32)
        for b in range(2):
            qpb = qpos[b * BLK:(b + 1) * BLK]
            nkv = KVT_A if b == 0 else KVT_B
            moff = 0 if b == 0 else KVT_A
            for kvt in range(nkv):
                kvp = perm[kvt * P:(kvt + 1) * P]
                m[:, moff + kvt, :] = (kvp[:, None] <= qpb[None, :])
        per_half[half] = dict(perm=perm, ckv=C, skv=Sg, mask=m.astype(BF))

    ones_pp = np.full((P, P), 1.0 / SA, BF)
    eye_pp = np.eye(P, dtype=np.float32)
    return dict(wq=wq, wk=wk, wv=wv, wo=wo, wgu=wgu, wgu8=wgu8, wd=wd,
                wd8=wd8, s_x=s_x, per_half=per_half, ones_pp=ones_pp,
                eye_pp=eye_pp, consts=consts)


def _core_in_map(shared, x, core):
    b, half = core // 2, core % 2
    ph = shared["per_half"][half]
    xT = x[b].T[:, ph["perm"]]                           # [H, SKV] permuted
    x_pack = np.ascontiguousarray(xT.reshape(KT, P, SKV).transpose(1, 0, 2))
    return {
        "x_kv": _q8(x_pack, shared["s_x"]),
        "x_qres": np.ascontiguousarray(x_pack[:, :, Q0:Q0 + T], np.float32),
        "ckv": ph["ckv"], "skv": ph["skv"], "mask": ph["mask"],
        "ones_pp": shared["ones_pp"], "eye_pp": shared["eye_pp"],
        "consts": shared["consts"],
        "wq": shared["wq"], "wk": shared["wk"], "wv": shared["wv"],
        "wo": shared["wo"], "wgu": shared["wgu"], "wgu8": shared["wgu8"],
        "wd": shared["wd"], "wd8": shared["wd8"],
    }


_NC = None


def kernel(**inputs):
    global _NC, LAST_RESULT
    if _NC is None:
        _NC = build_nc()
    nc = _NC

    shared = _prep_shared(inputs)
    x = np.asarray(inputs["hidden_states"], np.float32)    # [B,S,H]
    in_maps = [_core_in_map(shared, x, c) for c in range(N_CORES)]

    trace = bool(int(os.environ.get("BASS_TRACE", "0") or "0"))
    res = None
    for attempt in range(3):
        try:
            res = run_bass_kernel_spmd(nc, in_maps, core_ids=list(range(N_CORES)),
                                       trace=trace)
            break
        except Exception:
            # the axon terminal occasionally wedges transiently (LoadExecutable
            # failures); it recovers after a short idle
            if attempt == 2:
                raise
            import time
            time.sleep(90)
    LAST_RESULT = res

    out = np.empty((B, S, H), np.float32)
    for c in range(N_CORES):
        b, half = c // 2, c % 2
        qpos = _perm(half)[Q0:Q0 + T]
        y = res.results[c]["y"]                            # [128,16,512]
        out[b, qpos, :] = y.transpose(1, 0, 2).reshape(H, T).T
    return out
